# revision 1
# baseline (speedup 1.0000x reference)
"""Trainium2 Bass kernel for 3D conv-attention layer.

Reference (per (b,h,w) "site", D=32 positions, S=32 features):
  k,q,v = 1x1 conv of x [B,C,D,H,W] -> [B,S,D,H,W]
  scoresT[j,i] = sum_s q[s,j] k[s,i] / sqrt(S)   (per site)
  aT = softmax over i  (free dim of scoresT)
  o[s,j] = sum_i v[s,i] a[i,j];   y = x + Wo @ o + bo

Sharding: data-parallel over H across 8 cores.

Per-core strategy (per (b,h) chunk = 64 sites; halves of 32 sites):
  - Grid projections (tile_position col groups): K=64, M=32, N=256 matmuls
    place per-site [S,32] k/q/v tiles on distinct 32-partition blocks so that
    16 sites' attention matmuls run concurrently in the 128x128 PE array.
  - scoresT via 16 concurrent 32x32x32 matmuls; softmax over the free dim
    (exp without max-subtraction: |scores| <~ 7, exact-safe in fp32).
  - v->vT and aT->a via DVE 32x32 block transposes.
  - o via 16 concurrent matmuls -> [s, d] grid; output projection via 8
    packed matmuls (N=256); residual + out-proj bias + re-layout fused into
    per-row-group tensor_tensor ops reading PSUM directly.
  - All matmuls fp32 (exact): fp32r is ~2e-4 lossy on HW and only legal at
    tile_position column 0, which breaks the grid layout.
  - HW constraints honored: one sem-wait per instruction (Bacc event sems),
    and concurrent tile_position matmuls sharing a column group must write
    distinct PSUM banks (device crash otherwise).
"""

import math
from contextlib import ExitStack

import numpy as np

import concourse.bass as bass
import concourse.mybir as mybir
from concourse import bacc
import concourse.tile as tile
from concourse.bass_utils import run_bass_kernel_spmd

B, C, D, H, W = 4, 64, 32, 64, 64
S = C // 2  # 32
NCORES = 8
HS = H // NCORES
F32 = mybir.dt.float32
FR = mybir.dt.float32r

INV_SQRT_S = 1.0 / math.sqrt(S)


def mkap(base, part0, pcount, foff, fdims):
    """AP at partition block [part0, part0+pcount) of a tile, free offset foff,
    free dims [(step, count), ...] in the tile's flat free space."""
    full = base[...] if not isinstance(base, bass.AP) else base
    pstride = full.ap[0][0]
    return bass.AP(tensor=full.tensor,
                   offset=full.offset + part0 * pstride + foff,
                   ap=[[pstride, pcount]] + [list(d) for d in fdims])


def dap(handle, offset, dims):
    """Raw AP on a DRAM tensor: dims are [[step, count], ...] in elements."""
    full = handle[...]
    return bass.AP(tensor=full.tensor, offset=offset,
                   ap=[list(d) for d in dims])


def build_program(attn_dt=F32, proj_dt=FR):
    nc = bacc.Bacc()
    x_d = nc.declare_dram_parameter("x", [B, C, D, HS, W], F32, isOutput=False)
    # host-precomputed constant layouts (see make_in_maps)
    wk_d = nc.declare_dram_parameter("wkT", [C, S], F32, isOutput=False)
    wq_d = nc.declare_dram_parameter("wqT", [C, S], F32, isOutput=False)
    wv_d = nc.declare_dram_parameter("wvT", [C, S], F32, isOutput=False)
    wo_d = nc.declare_dram_parameter("woTr", [4 * S, C], F32, isOutput=False)
    bk_d = nc.declare_dram_parameter("bkr", [128, 1], F32, isOutput=False)
    bq_d = nc.declare_dram_parameter("bqr", [128, 1], F32, isOutput=False)
    bv_d = nc.declare_dram_parameter("bvr", [128, 1], F32, isOutput=False)
    bo_d = nc.declare_dram_parameter("boc", [C, 1], F32, isOutput=False)
    y_d = nc.declare_dram_parameter("y", [B, C, D, HS, W], F32, isOutput=True)

    def mm_dt(apx, dt):
        return apx.bitcast(dt) if dt != F32 else apx

    with tile.TileContext(nc) as tc, ExitStack() as ctx:
        const = ctx.enter_context(tc.tile_pool(name="const", bufs=1))
        xp = ctx.enter_context(tc.tile_pool(name="xp", bufs=3))
        att_ps = ctx.enter_context(tc.tile_pool(name="att_ps", bufs=1, space="PSUM"))
        sb = ctx.enter_context(tc.tile_pool(name="sb", bufs=3))
        outp = ctx.enter_context(tc.tile_pool(name="outp", bufs=2))

        # ---- constants (host-prelayouted; fp32r rounding copies on DVE) ----
        wkT_s = const.tile([C, S], F32, tag="wks")
        wqT_s = const.tile([C, S], F32, tag="wqs")
        wvT_s = const.tile([C, S], F32, tag="wvs")
        wkT = const.tile([C, S], proj_dt, tag="wk")
        wqT = const.tile([C, S], proj_dt, tag="wq")
        wvT = const.tile([C, S], proj_dt, tag="wv")
        for wt, ws, wd in ((wkT, wkT_s, wk_d), (wqT, wqT_s, wq_d),
                           (wvT, wvT_s, wv_d)):
            nc.sync.dma_start(out=ws[:, :], in_=wd[:, :])
            nc.vector.tensor_copy(out=wt[:, :], in_=ws[:, :])
        woT_s = const.tile([4 * S, C], F32, tag="wos")
        woT = const.tile([4 * S, C], proj_dt, tag="wo")
        nc.sync.dma_start(out=woT_s[:, :], in_=wo_d[:, :])
        nc.vector.tensor_copy(out=woT[:, :], in_=woT_s[:, :])
        bk_t = const.tile([128, 1], F32, tag="bk")
        bq_t = const.tile([128, 1], F32, tag="bq")
        bv_t = const.tile([128, 1], F32, tag="bv")
        for bt, bd in ((bk_t, bk_d), (bq_t, bq_d), (bv_t, bv_d)):
            nc.sync.dma_start(out=bt[:, :], in_=bd[:, :])
        bo_c = const.tile([C, 1], F32, tag="bo")
        nc.sync.dma_start(out=bo_c[:, :], in_=bo_d[:, :])

        for b in range(B):
            for h in range(HS):
                x_sb = xp.tile([C, D, W], F32, tag="x")
                # split by w-half: half 0's projections start after only
                # half the load; two DMA queues run in parallel per chunk
                nc.sync.dma_start(out=x_sb[:, :, 0:32],
                                  in_=x_d[b, :, :, h, 0:32])
                nc.sync.dma_start(out=x_sb[:, :, 32:64],
                                  in_=x_d[b, :, :, h, 32:64])
                y_sb = xp.tile([C, D, W], F32, tag="y")
                # x + bo precomputed (residual + out-proj bias in one term)
                xb_sb = xp.tile([C, D, W], F32, tag="xb")
                nc.gpsimd.tensor_scalar_add(xb_sb[:, :, :], x_sb[:, :, :],
                                            bo_c[:, :])
                if proj_dt != F32:
                    # x rounded to proj_dt for fp32r (ACT is idle)
                    x_r = xp.tile([C, D, W], proj_dt, tag="xr")
                    nc.scalar.activation(x_r[:, :, :], x_sb[:, :, :],
                                         mybir.ActivationFunctionType.Copy)
                else:
                    x_r = x_sb

                for half in range(2):
                    wb = 32 * half

                    # ---- projections into grid layouts ----
                    kg = att_ps.tile([128, 256], F32, tag="T0")
                    qg = att_ps.tile([128, 256], F32, tag="T1")
                    vg = att_ps.tile([128, 256], F32, tag="T2")
                    for r in range(4):
                        # sites idx%4==r: w = wb+r+4u, u=0..7; col = 32u+d
                        rhs = mkap(x_r, 0, C, wb + r, [[4, 8], [W, D]])
                        nc.tensor.matmul(kg[32 * r:32 * r + 32, :],
                                         wkT[:, :], rhs,
                                         start=True, stop=True,
                                         tile_position=(0, 32 * r))
                        nc.tensor.matmul(qg[32 * r:32 * r + 32, :],
                                         wqT[:, :], rhs,
                                         start=True, stop=True,
                                         tile_position=(0, 32 * r))
                    for c in range(4):
                        # sites (idx%16)//4==c: w = wb+16*s16+4c+jl
                        # col = 128*s16 + 32*jl + d
                        rhs = mkap(x_r, 0, C, wb + 4 * c,
                                   [[16, 2], [1, 4], [W, D]])
                        nc.tensor.matmul(vg[32 * c:32 * c + 32, :],
                                         wvT[:, :], rhs,
                                         start=True, stop=True,
                                         tile_position=(0, 32 * c))

                    # ---- PSUM -> SBUF with bias ----
                    k_sb = sb.tile([128, 256], F32, tag="k")
                    q_sb = sb.tile([128, 256], F32, tag="q")
                    v_sb = sb.tile([128, 256], F32, tag="v")
                    # k on DVE in parallel with q on ACT: scores need both,
                    # so splitting the drains across engines shortens the
                    # PE-critical path each half.
                    nc.vector.tensor_scalar_add(k_sb[:, :], kg[:, :], bk_t[:, :])
                    nc.scalar.activation(q_sb[:, :], qg[:, :],
                                         mybir.ActivationFunctionType.Identity,
                                         bias=bq_t[:, :])
                    nc.vector.tensor_scalar_add(v_sb[:, :], vg[:, :], bv_t[:, :])

                    vT_sb = sb.tile([128, 256], F32, tag="vT")
                    nc.vector.transpose(vT_sb[:, :], v_sb[:, :])

                    # PSUM banks: scores MMs sharing a column group from
                    # different row groups must land in different banks
                    # (HW crash otherwise) -> one bank tile per row group.
                    obank = []
                    for c in range(4):
                        ot = att_ps.tile([128, 64], F32, tag=f"T{c}")
                        obank.append(ot)

                    for s16 in range(2):
                        fo = 128 * s16
                        scb = []
                        for r in range(4):
                            st = att_ps.tile([128, 32], F32, tag=f"U{r}")
                            scb.append(st)
                        for j in range(16):
                            r, c = j % 4, j // 4
                            col = fo + 32 * c
                            nc.tensor.matmul(
                                scb[r][32 * c:32 * c + 32, 0:32],
                                mm_dt(q_sb[32 * r:32 * r + 32, col:col + 32], attn_dt),
                                mm_dt(k_sb[32 * r:32 * r + 32, col:col + 32], attn_dt),
                                start=True, stop=True,
                                tile_position=(32 * r, 32 * c))

                        # softmax over free dim
                        e_sb = sb.tile([128, 128], F32, tag="e")
                        for r in range(4):
                            nc.scalar.activation(e_sb[:, 32 * r:32 * r + 32],
                                                 scb[r][:, 0:32],
                                                 mybir.ActivationFunctionType.Exp,
                                                 scale=INV_SQRT_S)
                        den = sb.tile([128, 4], F32, tag="den")
                        nc.vector.reduce_sum(
                            out=den[:, :],
                            in_=mkap(e_sb, 0, 128, 0, [[32, 4], [1, 32]]),
                            axis=mybir.AxisListType.X)
                        rcp = sb.tile([128, 4], F32, tag="rcp")
                        nc.vector.reciprocal(rcp[:, :], den[:, :])
                        aT_sb = sb.tile([128, 128], F32, tag="aT")
                        nc.vector.tensor_tensor(
                            out=mkap(aT_sb, 0, 128, 0, [[32, 4], [1, 32]]),
                            in0=mkap(e_sb, 0, 128, 0, [[32, 4], [1, 32]]),
                            in1=mkap(rcp, 0, 128, 0, [[1, 4], [0, 32]]),
                            op=mybir.AluOpType.mult)
                        a_sb = sb.tile([128, 128], F32, tag="a")
                        nc.vector.transpose(a_sb[:, :], aT_sb[:, :])

                        # o-MM (site r,c): row group c, col group r ->
                        # bank by row group c; free offset 32*s16
                        for j in range(16):
                            r, c = j % 4, j // 4
                            nc.tensor.matmul(
                                obank[c][32 * r:32 * r + 32,
                                         32 * s16:32 * s16 + 32],
                                mm_dt(vT_sb[32 * c:32 * c + 32,
                                            fo + 32 * r:fo + 32 * r + 32], attn_dt),
                                mm_dt(a_sb[32 * c:32 * c + 32,
                                           32 * r:32 * r + 32], attn_dt),
                                start=True, stop=True,
                                tile_position=(32 * c, 32 * r))

                    # gather o banks -> o_sb [128, 256]: block (r, 128*s16+32*c)
                    o_sb = sb.tile([128, 256], proj_dt, tag="osb")
                    for c in range(4):
                        nc.scalar.activation(
                            mkap(o_sb, 0, 128, 32 * c, [[128, 2], [1, 32]]),
                            mkap(obank[c], 0, 128, 0, [[32, 2], [1, 32]]),
                            mybir.ActivationFunctionType.Copy)

                    # ---- output projection: 8 matmuls N=256 ----
                    # per-row-group banks (reuse U tags; scb dead by now)
                    opb = []
                    for r in range(4):
                        pt = att_ps.tile([C, 256], F32, tag=f"U{r}")
                        opb.append(pt)
                    for r in range(4):
                        for bh in range(2):
                            nc.tensor.matmul(
                                opb[r][32 * bh:32 * bh + 32, 0:256],
                                woT[32 * r:32 * r + 32, 32 * bh:32 * bh + 32],
                                o_sb[32 * r:32 * r + 32, :],
                                start=True, stop=True,
                                tile_position=(32 * r, 32 * bh))

                    # residual + re-layout: value (c_ch, w=wb+16s16+4c+r, dj)
                    # at opb[r] partition c_ch, free 128*s16 + 32*c + dj.
                    for r in range(4):
                        fdims_o = [[128, 2], [32, 4], [1, 32]]
                        fdims_x = [[16, 2], [4, 4], [W, D]]
                        in0 = mkap(opb[r], 0, C, 0, fdims_o)
                        x_in = mkap(xb_sb, 0, C, wb + r, fdims_x)
                        y_out = mkap(y_sb, 0, C, wb + r, fdims_x)
                        nc.vector.tensor_tensor(out=y_out, in0=in0, in1=x_in,
                                                op=mybir.AluOpType.add)

                nc.sync.dma_start(out=y_d[b, :, :, h, :], in_=y_sb[:, :, :])

    nc.finalize()
    return nc


_NC_CACHE = {}


def get_nc(key=("f32", "f32")):
    if key not in _NC_CACHE:
        dts = {"f32": F32, "fr": FR}
        _NC_CACHE[key] = build_program(attn_dt=dts[key[0]], proj_dt=dts[key[1]])
    return _NC_CACHE[key]


def make_in_maps(x, Wk, bk, Wq, bq, Wv, bv, Wo, bo):
    x = np.ascontiguousarray(np.asarray(x, dtype=np.float32))
    f = np.float32
    rep4 = lambda v: np.tile(np.asarray(v, f).reshape(-1), 4)[:, None]
    consts = {
        "wkT": np.ascontiguousarray(np.asarray(Wk, f).T),
        "wqT": np.ascontiguousarray(np.asarray(Wq, f).T),
        "wvT": np.ascontiguousarray(np.asarray(Wv, f).T),
        "woTr": np.ascontiguousarray(np.tile(np.asarray(Wo, f).T, (4, 1))),
        "bkr": np.ascontiguousarray(rep4(bk)),
        "bqr": np.ascontiguousarray(rep4(bq)),
        "bvr": np.ascontiguousarray(rep4(bv)),
        "boc": np.ascontiguousarray(np.asarray(bo, f)[:, None]),
    }
    in_maps = []
    for i in range(NCORES):
        m = {"x": np.ascontiguousarray(x[:, :, :, i * HS:(i + 1) * HS, :])}
        m.update(consts)
        in_maps.append(m)
    return in_maps


def gather(results):
    out = np.empty((B, C, D, H, W), dtype=np.float32)
    for i in range(NCORES):
        out[:, :, :, i * HS:(i + 1) * HS, :] = results[i]["y"]
    return out


def kernel(x, Wk, bk, Wq, bq, Wv, bv, Wo, bo):
    nc = get_nc()
    in_maps = make_in_maps(x, Wk, bk, Wq, bq, Wv, bv, Wo, bo)
    res = run_bass_kernel_spmd(nc, in_maps, core_ids=list(range(NCORES)))
    return gather(res.results)



# revision 24
# speedup vs baseline: 5.4157x; 5.4157x over previous
"""Trainium2 Bass kernel for 3D conv-attention layer (v2, bf16 + algebraic fusion).

Reference (per (b,h,w) "site", D=32 positions, S=32 features):
  k,q,v = 1x1 conv of x [B,C,D,H,W] -> [B,S,D,H,W]
  scoresT[j,i] = sum_s q[s,j] k[s,i] / sqrt(S)   (per site)
  aT = softmax over i  (free dim of scoresT)
  o[s,j] = sum_i v[s,i] a[i,j];   y = x + Wo @ o + bo

Key algebra (removes k/q projections AND the operand-colocation problem):
  scoresT/sqrt(S) = X~^T @ (G~ @ X)  per site, where
    G2 = Wq^T Wk / sqrt(S)  [C,C],  g = Wk^T bq / sqrt(S)  [C]
    G~ = [[G2],[g^T]] [C+1,C],  X~ = [X; ones] [C+1,D]
  (all j-only / const score terms cancel in the softmax over i;
   bv folds into a constant output bias since sum_i a[i,j] == 1:
   b* = Wo bv + bo, pre-added to x on the host.)

Cost-model-aware choices (TimelineSim charges matmuls by OUTPUT FREE SIZE
only, at 1 cycle/row for bf16; DVE/ACT ops by max free size):
  - everything bf16 into the PE, fp32 PSUM accumulation
  - P2 = G~X as 4 [65,512] matmuls/chunk (2048 rows)
  - vT = X^T Wv^T computed BY THE PE (16 matmuls, 4 sites each, 512 rows)
    -> no DVE StreamTranspose for v (StreamTranspose has no fast modes)
  - scores/o: 64 [32,32] matmuls each, stacked 4 sites per 128-partition
    PSUM tile so softmax runs as a few [128,512]-wide ops
  - single DVE block-transpose (aT -> a) per chunk
  - host pre-transposes x so every DMA is >=512B-contiguous per partition

Sharding: data-parallel over H (8 cores x 8 rows).
Per core: 32 chunks of (b,h), each 64 sites of [C=64, D=32].
"""

import math
from contextlib import ExitStack

import numpy as np
import ml_dtypes

import concourse.bass as bass
import concourse.mybir as mybir
from concourse import bacc
import concourse.tile as tile
from concourse.bass_utils import run_bass_kernel_spmd

B, C, D, H, W = 4, 64, 32, 64, 64
S = C // 2  # 32
NCORES = 8
HS = H // NCORES  # 8
F32 = mybir.dt.float32
BF16 = mybir.dt.bfloat16
FD = D * W  # 2048 free elems per chunk


def mkap(base, part0, pcount, foff, fdims):
    """AP at partition block [part0, part0+pcount) of a tile, free offset foff,
    free dims [(step, count), ...] in the tile's flat free space."""
    full = base[...] if not isinstance(base, bass.AP) else base
    pstride = full.ap[0][0]
    return bass.AP(tensor=full.tensor,
                   offset=full.offset + part0 * pstride + foff,
                   ap=[[pstride, pcount]] + [list(d) for d in fdims])


YDT = BF16  # output dtype (bf16 halves the store DMA)



def _copy(nc, eng, out, in_):
    if eng == "act":
        nc.scalar.activation(out, in_, mybir.ActivationFunctionType.Copy)
    elif eng == "dve":
        nc.vector.tensor_copy(out=out, in_=in_)
    else:
        nc.gpsimd.tensor_copy(out=out, in_=in_)


def _add(nc, eng, out, in0, in1):
    e = nc.vector if eng == "dve" else nc.gpsimd
    e.tensor_tensor(out=out, in0=in0, in1=in1, op=mybir.AluOpType.add)

def build_program(ydt=YDT, xt_bufs=3, xb_bufs=3, pdr_bufs=2, sb_bufs=2,
                  y_bufs=3, pj_bufs=2, vt_bufs=1, ab_bufs=1, yps_bufs=1,
                  probe=(), pd_eng=("act", "act", "dve", "act"),
                  vtsb_eng=("act", "act"), odr_eng=("act", "dve"),
                  fin_eng=("dve", "dve"), norm_eng="dve",
                  sc_order="tmaj"):
    nc = bacc.Bacc()
    xt_d = nc.declare_dram_parameter("xt", [B, HS, C + 1, FD], BF16,
                                     isOutput=False)
    xb_d = nc.declare_dram_parameter("xb", [B, HS, 128, 1024], BF16,
                                     isOutput=False)
    L_d = nc.declare_dram_parameter("L", [C, C + 1], BF16, isOutput=False)
    wv_d = nc.declare_dram_parameter("wvT", [C, S], BF16, isOutput=False)
    wo_d = nc.declare_dram_parameter("woTr", [128, C], BF16, isOutput=False)
    y_d = nc.declare_dram_parameter("y", [B, HS, 128, 1024], ydt,
                                    isOutput=True)

    EXP = mybir.ActivationFunctionType.Exp
    CPY = mybir.ActivationFunctionType.Copy

    with tile.TileContext(nc) as tc, ExitStack() as ctx:
        const = ctx.enter_context(tc.tile_pool(name="const", bufs=1))
        xtp = ctx.enter_context(tc.tile_pool(name="xtp", bufs=xt_bufs))
        xbp = ctx.enter_context(tc.tile_pool(name="xbp", bufs=xb_bufs))
        pdrp = ctx.enter_context(tc.tile_pool(name="pdrp", bufs=pdr_bufs))
        sbp = ctx.enter_context(tc.tile_pool(name="sbp", bufs=sb_bufs))
        ysp = ctx.enter_context(tc.tile_pool(name="ysp", bufs=y_bufs))
        pj_ps = ctx.enter_context(tc.tile_pool(name="pj_ps", bufs=pj_bufs,
                                               space="PSUM"))
        ab_ps = ctx.enter_context(tc.tile_pool(name="ab_ps", bufs=ab_bufs,
                                               space="PSUM"))
        o_ps = ctx.enter_context(tc.tile_pool(name="o_ps", bufs=vt_bufs,
                                              space="PSUM"))
        y_ps = ctx.enter_context(tc.tile_pool(name="y_ps", bufs=yps_bufs,
                                              space="PSUM"))

        # ---- constants ----
        L_sb = const.tile([C, C + 1], BF16, tag="L")
        nc.sync.dma_start(out=L_sb[:, :], in_=L_d[:, :])
        wv_sb = const.tile([C, S], BF16, tag="wv")
        nc.sync.dma_start(out=wv_sb[:, :], in_=wv_d[:, :])
        wo_sb = const.tile([128, C], BF16, tag="wo")
        nc.sync.dma_start(out=wo_sb[:, :], in_=wo_d[:, :])

        # ---------- software-pipelined chunk emission ----------
        # Engines run their queues in order, so the PE stream must never
        # queue o(i) (which waits on chunk i's softmax chain) ahead of
        # independent work.  Per iteration i the PE sees:
        #   proj/vT(i) | outproj(i-2) | scores(i) | o(i-1)
        # which keeps it busy while softmax(i-1) runs on ACT/DVE.
        chunks = [(b, h) for b in range(B) for h in range(HS)]
        n = len(chunks)
        st = {}  # chunk index -> dict of live tiles

        def stage_load_proj(i):
            b, h = chunks[i]
            s = st[i] = {}
            xt = s["xt"] = xtp.tile([C + 1, FD], BF16, tag="xt", name="xt")
            nc.sync.dma_start(out=xt[:, :], in_=xt_d[b, h, :, :])
            xb = s["xb"] = xbp.tile([128, 1024], BF16, tag="xb", name="xb")
            nc.sync.dma_start(out=xb[:, :], in_=xb_d[b, h, :, :])

            # A/B PSUM banks: cols 0-255 scoresT, 256-511 vT.  Same bank
            # from different tile columns is fine; the parity split keeps
            # same-column matmuls in alternating banks.
            tAB = s["tAB"] = (ab_ps.tile([128, 512], F32, tag="A", name="tA"),
                              ab_ps.tile([128, 512], F32, tag="B", name="tB"))

            def vt_mm(q):
                vt = tAB[q % 2]
                fo = 256 + 32 * (q // 2)
                nc.tensor.matmul(vt[:, fo:fo + 32],
                                 xt[0:C, 128 * q:128 * q + 128],
                                 wv_sb[:, :], start=True, stop=True,
                                 tile_position=(0, 0))
            s["vt_mm"] = vt_mm

            # P2 = G~ @ X [65, 512] x4, interleaved with vT = X^T Wv^T
            # matmuls to space same-column same-bank pairs apart.
            pdr = s["pdr"] = []
            for qd in range(4):
                pp = pj_ps.tile([C + 1, 512], F32, tag="pp", name="pp")
                nc.tensor.matmul(pp[:, :], L_sb[:, :],
                                 xt[0:C, 512 * qd:512 * qd + 512],
                                 start=True, stop=True, tile_position=(0, 0))
                vt_mm(2 * qd)
                vt_mm(2 * qd + 1)
                pd = pdrp.tile([C + 1, 512], BF16, tag=f"pd{qd}", name="pd")
                _copy(nc, pd_eng[qd], pd[:, :], pp[:, :])
                pdr.append(pd)

        def stage_scores_softmax(i):
            s = st[i]
            xt, pdr, tAB = s["xt"], s["pdr"], s["tAB"]
            # scoresT: site w=4t+bc -> parity bank by t%2, partitions 32*bc,
            # free 32*(t//2); remaining vT matmuls spaced among the scores.
            t_order = (list(range(0, 16, 2)) + list(range(1, 16, 2))
                       if sc_order == "parity" else list(range(16)))
            for ti, t in enumerate(t_order):
                sc = tAB[t % 2]
                fo = 32 * (t // 2)
                if ti < 8:
                    s["vt_mm"](8 + ti)
                for bc in range(4):
                    w = 4 * t + bc
                    nc.tensor.matmul(
                        sc[32 * bc:32 * bc + 32, fo:fo + 32],
                        xt[0:C + 1, 32 * w:32 * w + 32],
                        pdr[w // 16][0:C + 1,
                                     32 * (w % 16):32 * (w % 16) + 32],
                        start=True, stop=True,
                        tile_position=(0, 32 * bc))

            vtsb = s["vtsb"] = sbp.tile([128, 512], BF16, tag="vtsb",
                                        name="vtsb")
            _copy(nc, vtsb_eng[0],
                  mkap(vtsb, 0, 128, 0, [[64, 8], [1, 32]]),
                  tAB[0][:, 256:512])
            _copy(nc, vtsb_eng[1],
                  mkap(vtsb, 0, 128, 32, [[64, 8], [1, 32]]),
                  tAB[1][:, 256:512])

            # softmax over i (free dim)
            e_sb = sbp.tile([128, 512], BF16, tag="e", name="e_sb")
            nc.scalar.activation(
                mkap(e_sb, 0, 128, 0, [[64, 8], [1, 32]]),
                tAB[0][:, 0:256], EXP)
            nc.scalar.activation(
                mkap(e_sb, 0, 128, 32, [[64, 8], [1, 32]]),
                tAB[1][:, 0:256], EXP)
            den = sbp.tile([128, 16], F32, tag="den", name="den")
            nc.vector.reduce_sum(
                out=den[:, :],
                in_=mkap(e_sb, 0, 128, 0, [[32, 16], [1, 32]]),
                axis=mybir.AxisListType.X)
            rcp = sbp.tile([128, 16], F32, tag="rcp", name="rcp")
            nc.vector.reciprocal(rcp[:, :], den[:, :])
            atn = sbp.tile([128, 512], BF16, tag="atn", name="atn")
            neng = nc.gpsimd if norm_eng == "pool" else nc.vector
            neng.tensor_tensor(
                out=atn[:, :], in0=e_sb[:, :],
                in1=mkap(rcp, 0, 128, 0, [[1, 16], [0, 32]]),
                op=mybir.AluOpType.mult)
            a4 = s["a4"] = sbp.tile([128, 512], BF16, tag="a4", name="a4")
            nc.vector.transpose(a4[:, :], atn[:, :])

        def stage_o(i):
            s = st[i]
            vtsb, a4 = s["vtsb"], s["a4"]
            oab = (o_ps.tile([128, 256], F32, tag="oA", name="oa"),
                   o_ps.tile([128, 256], F32, tag="oB", name="ob"))
            for t in range(16):
                o_t = oab[t % 2]
                fo = 32 * (t // 2)
                for bc in range(4):
                    nc.tensor.matmul(
                        o_t[32 * bc:32 * bc + 32, fo:fo + 32],
                        vtsb[32 * bc:32 * bc + 32, 32 * t:32 * t + 32],
                        a4[32 * bc:32 * bc + 32, 32 * t:32 * t + 32],
                        start=True, stop=True,
                        tile_position=(32 * bc, 32 * bc))
            odr = s["odr"] = sbp.tile([128, 512], BF16, tag="odr",
                                      name="odr")
            _copy(nc, odr_eng[0],
                  mkap(odr, 0, 128, 0, [[64, 8], [1, 32]]),
                  oab[0][:, :])
            _copy(nc, odr_eng[1],
                  mkap(odr, 0, 128, 32, [[64, 8], [1, 32]]),
                  oab[1][:, :])

        def stage_out(i):
            b, h = chunks[i]
            s = st[i]
            odr, xb = s["odr"], s["xb"]
            yp0 = y_ps.tile([128, 512], F32, tag="y0", name="yp0")
            yp1 = y_ps.tile([128, 512], F32, tag="y1", name="yp1")
            for bc in range(4):
                yp = yp0 if bc < 2 else yp1
                nc.tensor.matmul(
                    yp[64 * (bc % 2):64 * (bc % 2) + 64, :],
                    wo_sb[32 * bc:32 * bc + 32, :],
                    odr[32 * bc:32 * bc + 32, :],
                    start=True, stop=True,
                    tile_position=(32 * bc, 64 * (bc % 2)))
            y_sb = ysp.tile([128, 1024], ydt, tag="y", name="y_sb")
            _add(nc, fin_eng[0], y_sb[:, 0:512], yp0[:, :], xb[:, 0:512])
            _add(nc, fin_eng[1], y_sb[:, 512:1024], yp1[:, :],
                 xb[:, 512:1024])
            nc.sync.dma_start(out=y_d[b, h, :, :], in_=y_sb[:, :])
            del st[i]

        for i in range(n + 2):
            if i < n:
                stage_load_proj(i)
            if i >= 2:
                stage_out(i - 2)
            if i < n:
                stage_scores_softmax(i)
            if i >= 1 and i - 1 < n:
                stage_o(i - 1)

    nc.finalize()
    return nc


_NC_CACHE = {}


def get_nc():
    if "nc" not in _NC_CACHE:
        _NC_CACHE["nc"] = build_program()
    return _NC_CACHE["nc"]


def make_in_maps(x, Wk, bk, Wq, bq, Wv, bv, Wo, bo):
    f = np.float32
    bf = ml_dtypes.bfloat16
    x = np.asarray(x, f)
    Wk, bk = np.asarray(Wk, f), np.asarray(bk, f)
    Wq, bq = np.asarray(Wq, f), np.asarray(bq, f)
    Wv, bv = np.asarray(Wv, f), np.asarray(bv, f)
    Wo, bo = np.asarray(Wo, f), np.asarray(bo, f)

    isq = 1.0 / math.sqrt(S)
    G2 = (Wq.T @ Wk) * isq                     # [C, C]
    g = (Wk.T @ bq) * isq                      # [C]
    Gt = np.concatenate([G2, g[None, :]], 0)   # [C+1, C]
    L = np.ascontiguousarray(Gt.T).astype(bf)  # [C, C+1] lhsT
    wvT = np.ascontiguousarray(Wv.T).astype(bf)          # [C, S]
    woTr = np.ascontiguousarray(np.tile(Wo.T, (4, 1))).astype(bf)  # [128, C]
    bst = Wo @ bv + bo                         # [C]

    # xt: [B, H, C+1, W*D] bf16, free index = 32*w + d, ones row at c=C
    xt_full = np.empty((B, H, C + 1, FD), dtype=bf)
    xw = np.transpose(x, (0, 3, 1, 4, 2))      # [B, H, C, W, D]
    xt_full[:, :, :C, :] = xw.reshape(B, H, C, FD).astype(bf)
    xt_full[:, :, C, :] = np.float32(1.0)

    # xb: [B, H, 128, 1024] bf16:
    #   partition 64*sg + c, free 512*hf + 32*t + d, site w = 4*t + 2*hf + sg
    xbv = x + bst[None, :, None, None, None]
    # [B, H, C, W, D] -> split w = 4*t + 2*hf + sg -> [B, H, sg, c, hf, t, d]
    xb6 = np.transpose(xbv, (0, 3, 1, 4, 2)).reshape(B, H, C, W // 4, 2, 2, D)
    # axes: b, h, c, t, hf, sg, d -> want [b, h, sg, c, hf, t, d]
    xb_full = np.ascontiguousarray(
        np.transpose(xb6, (0, 1, 5, 2, 4, 3, 6))).reshape(
            B, H, 128, 1024).astype(bf)

    in_maps = []
    for i in range(NCORES):
        sl = slice(i * HS, (i + 1) * HS)
        m = {
            "xt": np.ascontiguousarray(xt_full[:, sl]),
            "xb": np.ascontiguousarray(xb_full[:, sl]),
            "L": L, "wvT": wvT, "woTr": woTr,
        }
        in_maps.append(m)
    return in_maps


def gather(results):
    out = np.empty((B, C, D, H, W), dtype=np.float32)
    for i in range(NCORES):
        yr = np.asarray(results[i]["y"], dtype=np.float32)  # [B,HS,128,1024]
        y7 = yr.reshape(B, HS, 2, 64, 2, 16, 32)  # b,h,sg,c,hf,t,d
        # -> [B, C, D, HS, t, hf, sg] then w = 4t + 2hf + sg
        yw = np.transpose(y7, (0, 3, 6, 1, 5, 4, 2)).reshape(
            B, 64, 32, HS, W)
        out[:, :, :, i * HS:(i + 1) * HS, :] = yw
    return out


def kernel(x, Wk, bk, Wq, bq, Wv, bv, Wo, bo):
    nc = get_nc()
    in_maps = make_in_maps(x, Wk, bk, Wq, bq, Wv, bv, Wo, bo)
    res = run_bass_kernel_spmd(nc, in_maps, core_ids=list(range(NCORES)))
    return gather(res.results)


# revision 28
# speedup vs baseline: 6.0490x; 1.1169x over previous
"""Trainium2 Bass kernel for 3D conv-attention layer (v2, bf16 + algebraic fusion).

Reference (per (b,h,w) "site", D=32 positions, S=32 features):
  k,q,v = 1x1 conv of x [B,C,D,H,W] -> [B,S,D,H,W]
  scoresT[j,i] = sum_s q[s,j] k[s,i] / sqrt(S)   (per site)
  aT = softmax over i  (free dim of scoresT)
  o[s,j] = sum_i v[s,i] a[i,j];   y = x + Wo @ o + bo

Key algebra (removes k/q projections AND the operand-colocation problem):
  scoresT/sqrt(S) = X~^T @ (G~ @ X)  per site, where
    G2 = Wq^T Wk / sqrt(S)  [C,C],  g = Wk^T bq / sqrt(S)  [C]
    G~ = [[G2],[g^T]] [C+1,C],  X~ = [X; ones] [C+1,D]
  (all j-only / const score terms cancel in the softmax over i;
   bv folds into a constant output bias since sum_i a[i,j] == 1:
   b* = Wo bv + bo, pre-added to x on the host.)

Cost-model-aware choices (TimelineSim charges matmuls by OUTPUT FREE SIZE
only, at 1 cycle/row for bf16; DVE/ACT ops by max free size):
  - everything bf16 into the PE, fp32 PSUM accumulation
  - P2 = G~X as 4 [65,512] matmuls/chunk (2048 rows)
  - vT = X^T Wv^T computed BY THE PE (16 matmuls, 4 sites each, 512 rows)
    -> no DVE StreamTranspose for v (StreamTranspose has no fast modes)
  - scores/o: 64 [32,32] matmuls each, stacked 4 sites per 128-partition
    PSUM tile so softmax runs as a few [128,512]-wide ops
  - single DVE block-transpose (aT -> a) per chunk
  - host pre-transposes x so every DMA is >=512B-contiguous per partition

Sharding: data-parallel over H (8 cores x 8 rows).
Per core: 32 chunks of (b,h), each 64 sites of [C=64, D=32].
"""

import math
from contextlib import ExitStack

import numpy as np
import ml_dtypes

import concourse.bass as bass
import concourse.mybir as mybir
from concourse import bacc
import concourse.tile as tile
from concourse.bass_utils import run_bass_kernel_spmd

B, C, D, H, W = 4, 64, 32, 64, 64
S = C // 2  # 32
NCORES = 8
HS = H // NCORES  # 8
F32 = mybir.dt.float32
BF16 = mybir.dt.bfloat16
FD = D * W  # 2048 free elems per chunk


def mkap(base, part0, pcount, foff, fdims):
    """AP at partition block [part0, part0+pcount) of a tile, free offset foff,
    free dims [(step, count), ...] in the tile's flat free space."""
    full = base[...] if not isinstance(base, bass.AP) else base
    pstride = full.ap[0][0]
    return bass.AP(tensor=full.tensor,
                   offset=full.offset + part0 * pstride + foff,
                   ap=[[pstride, pcount]] + [list(d) for d in fdims])


YDT = BF16  # output dtype (bf16 halves the store DMA)



def _copy(nc, eng, out, in_):
    if eng == "act":
        nc.scalar.activation(out, in_, mybir.ActivationFunctionType.Copy)
    elif eng == "dve":
        nc.vector.tensor_copy(out=out, in_=in_)
    else:
        nc.gpsimd.tensor_copy(out=out, in_=in_)


def _add(nc, eng, out, in0, in1):
    e = nc.vector if eng == "dve" else nc.gpsimd
    e.tensor_tensor(out=out, in0=in0, in1=in1, op=mybir.AluOpType.add)

def build_program(ydt=YDT, xt_bufs=3, xb_bufs=3, pdr_bufs=2, sb_bufs=2,
                  y_bufs=3, pj_bufs=2, vt_bufs=1, ab_bufs=1, yps_bufs=1,
                  probe=(), pd_eng=("act", "act", "dve", "act"),
                  vtsb_eng=("act", "act"), odr_eng=("act", "dve"),
                  fin_eng=("dve", "dve"), norm_eng="dve",
                  sc_order="tmaj", layout="single"):
    nc = bacc.Bacc()
    xt_d = nc.declare_dram_parameter("xt", [B, HS, C + 1, FD], BF16,
                                     isOutput=False)
    xb_d = nc.declare_dram_parameter("xb", [B, HS, 128, 1024], BF16,
                                     isOutput=False)
    L_d = nc.declare_dram_parameter("L", [C, C + 1], BF16, isOutput=False)
    wv_d = nc.declare_dram_parameter("wvT", [C, S], BF16, isOutput=False)
    wo_d = nc.declare_dram_parameter("woTr", [128, C], BF16, isOutput=False)
    y_d = nc.declare_dram_parameter("y", [B, HS, 128, 1024], ydt,
                                    isOutput=True)

    EXP = mybir.ActivationFunctionType.Exp
    CPY = mybir.ActivationFunctionType.Copy

    with tile.TileContext(nc) as tc, ExitStack() as ctx:
        const = ctx.enter_context(tc.tile_pool(name="const", bufs=1))
        xtp = ctx.enter_context(tc.tile_pool(name="xtp", bufs=xt_bufs))
        xbp = ctx.enter_context(tc.tile_pool(name="xbp", bufs=xb_bufs))
        pdrp = ctx.enter_context(tc.tile_pool(name="pdrp", bufs=pdr_bufs))
        sbp = ctx.enter_context(tc.tile_pool(name="sbp", bufs=sb_bufs))
        ysp = ctx.enter_context(tc.tile_pool(name="ysp", bufs=y_bufs))
        if layout == "single":
            pj_bufs = max(pj_bufs, 3)
        pj_ps = ctx.enter_context(tc.tile_pool(name="pj_ps", bufs=pj_bufs,
                                               space="PSUM"))
        ab_ps = ctx.enter_context(tc.tile_pool(name="ab_ps", bufs=ab_bufs,
                                               space="PSUM"))
        o_ps = ctx.enter_context(tc.tile_pool(name="o_ps", bufs=vt_bufs,
                                              space="PSUM"))
        y_ps = ctx.enter_context(tc.tile_pool(name="y_ps", bufs=yps_bufs,
                                              space="PSUM"))

        # ---- constants ----
        L_sb = const.tile([C, C + 1], BF16, tag="L")
        nc.sync.dma_start(out=L_sb[:, :], in_=L_d[:, :])
        wv_sb = const.tile([C, S], BF16, tag="wv")
        nc.sync.dma_start(out=wv_sb[:, :], in_=wv_d[:, :])
        wo_sb = const.tile([128, C], BF16, tag="wo")
        nc.sync.dma_start(out=wo_sb[:, :], in_=wo_d[:, :])

        # ---------- software-pipelined chunk emission ----------
        # Engines run their queues in order, so the PE stream must never
        # queue o(i) (which waits on chunk i's softmax chain) ahead of
        # independent work.  Per iteration i the PE sees:
        #   proj/vT(i) | outproj(i-2) | scores(i) | o(i-1)
        # which keeps it busy while softmax(i-1) runs on ACT/DVE.
        chunks = [(b, h) for b in range(B) for h in range(HS)]
        n = len(chunks)
        st = {}  # chunk index -> dict of live tiles

        def stage_load_proj(i):
            b, h = chunks[i]
            s = st[i] = {}
            xt = s["xt"] = xtp.tile([C + 1, FD], BF16, tag="xt", name="xt")
            nc.sync.dma_start(out=xt[:, :], in_=xt_d[b, h, :, :])
            xb = s["xb"] = xbp.tile([128, 1024], BF16, tag="xb", name="xb")
            nc.sync.dma_start(out=xb[:, :], in_=xb_d[b, h, :, :])

            # A/B PSUM banks: cols 0-255 scoresT, 256-511 vT.  Same bank
            # from different tile columns is fine; the parity split keeps
            # same-column matmuls in alternating banks.
            if layout == "single":
                tS = ab_ps.tile([128, 512], F32, tag="A", name="tS")
                tV = ab_ps.tile([128, 512], F32, tag="V", name="tV")
                s["tAB"] = (tS, tS)
                s["tV"] = tV

                def vt_mm(q):
                    nc.tensor.matmul(tV[:, 32 * q:32 * q + 32],
                                     xt[0:C, 128 * q:128 * q + 128],
                                     wv_sb[:, :], start=True, stop=True,
                                     tile_position=(0, 0))
            else:
                s["tAB"] = (ab_ps.tile([128, 512], F32, tag="A", name="tA"),
                            ab_ps.tile([128, 512], F32, tag="B", name="tB"))

                def vt_mm(q):
                    vt = s["tAB"][q % 2]
                    fo = 256 + 32 * (q // 2)
                    nc.tensor.matmul(vt[:, fo:fo + 32],
                                     xt[0:C, 128 * q:128 * q + 128],
                                     wv_sb[:, :], start=True, stop=True,
                                     tile_position=(0, 0))
            tAB = s["tAB"]
            s["vt_mm"] = vt_mm

            # P2 = G~ @ X [65, 512] x4, interleaved with vT = X^T Wv^T
            # matmuls to space same-column same-bank pairs apart.
            pdr = s["pdr"] = []
            for qd in range(4):
                pp = pj_ps.tile([C + 1, 512], F32, tag="pp", name="pp")
                nc.tensor.matmul(pp[:, :], L_sb[:, :],
                                 xt[0:C, 512 * qd:512 * qd + 512],
                                 start=True, stop=True, tile_position=(0, 0))
                vt_mm(2 * qd)
                vt_mm(2 * qd + 1)
                pd = pdrp.tile([C + 1, 512], BF16, tag=f"pd{qd}", name="pd")
                _copy(nc, pd_eng[qd], pd[:, :], pp[:, :])
                pdr.append(pd)

        def stage_scores_softmax(i):
            s = st[i]
            xt, pdr, tAB = s["xt"], s["pdr"], s["tAB"]
            # scoresT: site w=4t+bc -> parity bank by t%2, partitions 32*bc,
            # free 32*(t//2); remaining vT matmuls spaced among the scores.
            t_order = (list(range(0, 16, 2)) + list(range(1, 16, 2))
                       if sc_order == "parity" else list(range(16)))
            nvt_pre = 8 if layout != "single" else 12
            for ti, t in enumerate(t_order):
                sc = tAB[t % 2]
                fo = 32 * t if layout == "single" else 32 * (t // 2)
                if ti < nvt_pre:
                    s["vt_mm"]((16 - nvt_pre) + ti)
                for bc in range(4):
                    w = 4 * t + bc
                    nc.tensor.matmul(
                        sc[32 * bc:32 * bc + 32, fo:fo + 32],
                        xt[0:C + 1, 32 * w:32 * w + 32],
                        pdr[w // 16][0:C + 1,
                                     32 * (w % 16):32 * (w % 16) + 32],
                        start=True, stop=True,
                        tile_position=(0, 32 * bc))

            vtsb = s["vtsb"] = sbp.tile([128, 512], BF16, tag="vtsb",
                                        name="vtsb")
            e_sb = sbp.tile([128, 512], BF16, tag="e", name="e_sb")
            if layout == "single":
                _copy(nc, vtsb_eng[0], vtsb[:, :], s["tV"][:, :])
                nc.scalar.activation(e_sb[:, :], tAB[0][:, :], EXP)
            else:
                _copy(nc, vtsb_eng[0],
                      mkap(vtsb, 0, 128, 0, [[64, 8], [1, 32]]),
                      tAB[0][:, 256:512])
                _copy(nc, vtsb_eng[1],
                      mkap(vtsb, 0, 128, 32, [[64, 8], [1, 32]]),
                      tAB[1][:, 256:512])
                nc.scalar.activation(
                    mkap(e_sb, 0, 128, 0, [[64, 8], [1, 32]]),
                    tAB[0][:, 0:256], EXP)
                nc.scalar.activation(
                    mkap(e_sb, 0, 128, 32, [[64, 8], [1, 32]]),
                    tAB[1][:, 0:256], EXP)
            den = sbp.tile([128, 16], F32, tag="den", name="den")
            nc.vector.reduce_sum(
                out=den[:, :],
                in_=mkap(e_sb, 0, 128, 0, [[32, 16], [1, 32]]),
                axis=mybir.AxisListType.X)
            atn = sbp.tile([128, 512], BF16, tag="atn", name="atn")
            if norm_eng == "div":
                nc.vector.tensor_tensor(
                    out=atn[:, :], in0=e_sb[:, :],
                    in1=mkap(den, 0, 128, 0, [[1, 16], [0, 32]]),
                    op=mybir.AluOpType.divide)
            elif norm_eng == "pooldiv":
                nc.gpsimd.tensor_tensor(
                    out=atn[:, :], in0=e_sb[:, :],
                    in1=mkap(den, 0, 128, 0, [[1, 16], [0, 32]]),
                    op=mybir.AluOpType.divide)
            else:
                rcp = sbp.tile([128, 16], F32, tag="rcp", name="rcp")
                nc.vector.reciprocal(rcp[:, :], den[:, :])
                neng = nc.gpsimd if norm_eng == "pool" else nc.vector
                neng.tensor_tensor(
                    out=atn[:, :], in0=e_sb[:, :],
                    in1=mkap(rcp, 0, 128, 0, [[1, 16], [0, 32]]),
                    op=mybir.AluOpType.mult)
            a4 = s["a4"] = sbp.tile([128, 512], BF16, tag="a4", name="a4")
            nc.vector.transpose(a4[:, :], atn[:, :])

        def stage_o(i):
            s = st[i]
            vtsb, a4 = s["vtsb"], s["a4"]
            odr = s["odr"] = sbp.tile([128, 512], BF16, tag="odr",
                                      name="odr")
            if layout == "single":
                oS = o_ps.tile([128, 512], F32, tag="oA", name="oS")
                for t in range(16):
                    for bc in range(4):
                        nc.tensor.matmul(
                            oS[32 * bc:32 * bc + 32, 32 * t:32 * t + 32],
                            vtsb[32 * bc:32 * bc + 32, 32 * t:32 * t + 32],
                            a4[32 * bc:32 * bc + 32, 32 * t:32 * t + 32],
                            start=True, stop=True,
                            tile_position=(32 * bc, 32 * bc))
                _copy(nc, odr_eng[0], odr[:, :], oS[:, :])
            else:
                oab = (o_ps.tile([128, 256], F32, tag="oA", name="oa"),
                       o_ps.tile([128, 256], F32, tag="oB", name="ob"))
                for t in range(16):
                    o_t = oab[t % 2]
                    fo = 32 * (t // 2)
                    for bc in range(4):
                        nc.tensor.matmul(
                            o_t[32 * bc:32 * bc + 32, fo:fo + 32],
                            vtsb[32 * bc:32 * bc + 32, 32 * t:32 * t + 32],
                            a4[32 * bc:32 * bc + 32, 32 * t:32 * t + 32],
                            start=True, stop=True,
                            tile_position=(32 * bc, 32 * bc))
                _copy(nc, odr_eng[0],
                      mkap(odr, 0, 128, 0, [[64, 8], [1, 32]]),
                      oab[0][:, :])
                _copy(nc, odr_eng[1],
                      mkap(odr, 0, 128, 32, [[64, 8], [1, 32]]),
                      oab[1][:, :])

        def stage_out(i):
            b, h = chunks[i]
            s = st[i]
            odr, xb = s["odr"], s["xb"]
            yp0 = y_ps.tile([128, 512], F32, tag="y0", name="yp0")
            yp1 = y_ps.tile([128, 512], F32, tag="y1", name="yp1")
            for bc in range(4):
                yp = yp0 if bc < 2 else yp1
                nc.tensor.matmul(
                    yp[64 * (bc % 2):64 * (bc % 2) + 64, :],
                    wo_sb[32 * bc:32 * bc + 32, :],
                    odr[32 * bc:32 * bc + 32, :],
                    start=True, stop=True,
                    tile_position=(32 * bc, 64 * (bc % 2)))
            y_sb = ysp.tile([128, 1024], ydt, tag="y", name="y_sb")
            _add(nc, fin_eng[0], y_sb[:, 0:512], yp0[:, :], xb[:, 0:512])
            _add(nc, fin_eng[1], y_sb[:, 512:1024], yp1[:, :],
                 xb[:, 512:1024])
            nc.sync.dma_start(out=y_d[b, h, :, :], in_=y_sb[:, :])
            del st[i]

        for i in range(n + 2):
            if i < n:
                stage_load_proj(i)
            if i >= 2:
                stage_out(i - 2)
            if i < n:
                stage_scores_softmax(i)
            if i >= 1 and i - 1 < n:
                stage_o(i - 1)

    nc.finalize()
    return nc


_NC_CACHE = {}


def get_nc():
    if "nc" not in _NC_CACHE:
        _NC_CACHE["nc"] = build_program()
    return _NC_CACHE["nc"]


def make_in_maps(x, Wk, bk, Wq, bq, Wv, bv, Wo, bo):
    f = np.float32
    bf = ml_dtypes.bfloat16
    x = np.asarray(x, f)
    Wk, bk = np.asarray(Wk, f), np.asarray(bk, f)
    Wq, bq = np.asarray(Wq, f), np.asarray(bq, f)
    Wv, bv = np.asarray(Wv, f), np.asarray(bv, f)
    Wo, bo = np.asarray(Wo, f), np.asarray(bo, f)

    isq = 1.0 / math.sqrt(S)
    G2 = (Wq.T @ Wk) * isq                     # [C, C]
    g = (Wk.T @ bq) * isq                      # [C]
    Gt = np.concatenate([G2, g[None, :]], 0)   # [C+1, C]
    L = np.ascontiguousarray(Gt.T).astype(bf)  # [C, C+1] lhsT
    wvT = np.ascontiguousarray(Wv.T).astype(bf)          # [C, S]
    woTr = np.ascontiguousarray(np.tile(Wo.T, (4, 1))).astype(bf)  # [128, C]
    bst = Wo @ bv + bo                         # [C]

    # xt: [B, H, C+1, W*D] bf16, free index = 32*w + d, ones row at c=C
    xt_full = np.empty((B, H, C + 1, FD), dtype=bf)
    xw = np.transpose(x, (0, 3, 1, 4, 2))      # [B, H, C, W, D]
    xt_full[:, :, :C, :] = xw.reshape(B, H, C, FD).astype(bf)
    xt_full[:, :, C, :] = np.float32(1.0)

    # xb: [B, H, 128, 1024] bf16:
    #   partition 64*sg + c, free 512*hf + 32*t + d, site w = 4*t + 2*hf + sg
    xbv = x + bst[None, :, None, None, None]
    # [B, H, C, W, D] -> split w = 4*t + 2*hf + sg -> [B, H, sg, c, hf, t, d]
    xb6 = np.transpose(xbv, (0, 3, 1, 4, 2)).reshape(B, H, C, W // 4, 2, 2, D)
    # axes: b, h, c, t, hf, sg, d -> want [b, h, sg, c, hf, t, d]
    xb_full = np.ascontiguousarray(
        np.transpose(xb6, (0, 1, 5, 2, 4, 3, 6))).reshape(
            B, H, 128, 1024).astype(bf)

    in_maps = []
    for i in range(NCORES):
        sl = slice(i * HS, (i + 1) * HS)
        m = {
            "xt": np.ascontiguousarray(xt_full[:, sl]),
            "xb": np.ascontiguousarray(xb_full[:, sl]),
            "L": L, "wvT": wvT, "woTr": woTr,
        }
        in_maps.append(m)
    return in_maps


def gather(results):
    out = np.empty((B, C, D, H, W), dtype=np.float32)
    for i in range(NCORES):
        yr = np.asarray(results[i]["y"], dtype=np.float32)  # [B,HS,128,1024]
        y7 = yr.reshape(B, HS, 2, 64, 2, 16, 32)  # b,h,sg,c,hf,t,d
        # -> [B, C, D, HS, t, hf, sg] then w = 4t + 2hf + sg
        yw = np.transpose(y7, (0, 3, 6, 1, 5, 4, 2)).reshape(
            B, 64, 32, HS, W)
        out[:, :, :, i * HS:(i + 1) * HS, :] = yw
    return out


def kernel(x, Wk, bk, Wq, bq, Wv, bv, Wo, bo):
    nc = get_nc()
    in_maps = make_in_maps(x, Wk, bk, Wq, bq, Wv, bv, Wo, bo)
    res = run_bass_kernel_spmd(nc, in_maps, core_ids=list(range(NCORES)))
    return gather(res.results)


# revision 36
# speedup vs baseline: 6.3518x; 1.0501x over previous
"""Trainium2 Bass kernel for 3D conv-attention layer (v2, bf16 + algebraic fusion).

Reference (per (b,h,w) "site", D=32 positions, S=32 features):
  k,q,v = 1x1 conv of x [B,C,D,H,W] -> [B,S,D,H,W]
  scoresT[j,i] = sum_s q[s,j] k[s,i] / sqrt(S)   (per site)
  aT = softmax over i  (free dim of scoresT)
  o[s,j] = sum_i v[s,i] a[i,j];   y = x + Wo @ o + bo

Key algebra (removes k/q projections AND the operand-colocation problem):
  scoresT/sqrt(S) = X~^T @ (G~ @ X)  per site, where
    G2 = Wq^T Wk / sqrt(S)  [C,C],  g = Wk^T bq / sqrt(S)  [C]
    G~ = [[G2],[g^T]] [C+1,C],  X~ = [X; ones] [C+1,D]
  (all j-only / const score terms cancel in the softmax over i;
   bv folds into a constant output bias since sum_i a[i,j] == 1:
   b* = Wo bv + bo, pre-added to x on the host.)

Cost-model-aware choices (TimelineSim charges matmuls by OUTPUT FREE SIZE
only, at 1 cycle/row for bf16; DVE/ACT ops by max free size):
  - everything bf16 into the PE, fp32 PSUM accumulation
  - P2 = G~X as 4 [65,512] matmuls/chunk (2048 rows)
  - vT = X^T Wv^T computed BY THE PE (16 matmuls, 4 sites each, 512 rows)
    -> no DVE StreamTranspose for v (StreamTranspose has no fast modes)
  - scores/o: 64 [32,32] matmuls each, stacked 4 sites per 128-partition
    PSUM tile so softmax runs as a few [128,512]-wide ops
  - single DVE block-transpose (aT -> a) per chunk
  - host pre-transposes x so every DMA is >=512B-contiguous per partition

Sharding: data-parallel over H (8 cores x 8 rows).
Per core: 32 chunks of (b,h), each 64 sites of [C=64, D=32].
"""

import math
from contextlib import ExitStack

import numpy as np
import ml_dtypes

import concourse.bass as bass
import concourse.mybir as mybir
from concourse import bacc
import concourse.tile as tile
from concourse.bass_utils import run_bass_kernel_spmd

B, C, D, H, W = 4, 64, 32, 64, 64
S = C // 2  # 32
NCORES = 8
HS = H // NCORES  # 8
F32 = mybir.dt.float32
BF16 = mybir.dt.bfloat16
FD = D * W  # 2048 free elems per chunk


def mkap(base, part0, pcount, foff, fdims):
    """AP at partition block [part0, part0+pcount) of a tile, free offset foff,
    free dims [(step, count), ...] in the tile's flat free space."""
    full = base[...] if not isinstance(base, bass.AP) else base
    pstride = full.ap[0][0]
    return bass.AP(tensor=full.tensor,
                   offset=full.offset + part0 * pstride + foff,
                   ap=[[pstride, pcount]] + [list(d) for d in fdims])


YDT = BF16  # output dtype (bf16 halves the store DMA)



def _copy(nc, eng, out, in_):
    if eng == "act":
        nc.scalar.activation(out, in_, mybir.ActivationFunctionType.Copy)
    elif eng == "dve":
        nc.vector.tensor_copy(out=out, in_=in_)
    else:
        nc.gpsimd.tensor_copy(out=out, in_=in_)


def _add(nc, eng, out, in0, in1):
    e = nc.vector if eng == "dve" else nc.gpsimd
    e.tensor_tensor(out=out, in0=in0, in1=in1, op=mybir.AluOpType.add)

def build_program(ydt=YDT, xt_bufs=4, xb_bufs=3, pdr_bufs=2, sb_bufs=3,
                  y_bufs=3, pj_bufs=2, vt_bufs=1, ab_bufs=1, yps_bufs=1,
                  probe=(), pd_eng=("act", "act", "act", "act"),
                  vtsb_eng=("dve", "act"), odr_eng=("act", "dve"),
                  fin_eng=("dve", "dve"), norm_eng="dve",
                  sc_order="tmaj", layout="single"):
    nc = bacc.Bacc()
    xt_d = nc.declare_dram_parameter("xt", [B, HS, C + 1, FD], BF16,
                                     isOutput=False)
    xb_d = nc.declare_dram_parameter("xb", [B, HS, 128, 1024], BF16,
                                     isOutput=False)
    L_d = nc.declare_dram_parameter("L", [C, C + 1], BF16, isOutput=False)
    wu_d = nc.declare_dram_parameter("wuT", [C, C], BF16, isOutput=False)
    y_d = nc.declare_dram_parameter("y", [B, HS, 128, 1024], ydt,
                                    isOutput=True)

    EXP = mybir.ActivationFunctionType.Exp
    CPY = mybir.ActivationFunctionType.Copy

    with tile.TileContext(nc) as tc, ExitStack() as ctx:
        const = ctx.enter_context(tc.tile_pool(name="const", bufs=1))
        xtp = ctx.enter_context(tc.tile_pool(name="xtp", bufs=xt_bufs))
        xbp = ctx.enter_context(tc.tile_pool(name="xbp", bufs=xb_bufs))
        pdrp = ctx.enter_context(tc.tile_pool(name="pdrp", bufs=pdr_bufs))
        sbp = ctx.enter_context(tc.tile_pool(name="sbp", bufs=sb_bufs))
        ysp = ctx.enter_context(tc.tile_pool(name="ysp", bufs=y_bufs))
        if layout == "single":
            pj_bufs = max(pj_bufs, 3)
        pj_ps = ctx.enter_context(tc.tile_pool(name="pj_ps", bufs=pj_bufs,
                                               space="PSUM"))
        ab_ps = ctx.enter_context(tc.tile_pool(name="ab_ps", bufs=ab_bufs,
                                               space="PSUM"))
        o_ps = ctx.enter_context(tc.tile_pool(name="o_ps", bufs=vt_bufs,
                                              space="PSUM"))
        y_ps = ctx.enter_context(tc.tile_pool(name="y_ps", bufs=yps_bufs,
                                              space="PSUM"))

        # ---- constants ----
        L_sb = const.tile([C, C + 1], BF16, tag="L")
        nc.sync.dma_start(out=L_sb[:, :], in_=L_d[:, :])
        wu_sb = const.tile([C, C], BF16, tag="wu")
        nc.sync.dma_start(out=wu_sb[:, :], in_=wu_d[:, :])

        # ---------- software-pipelined chunk emission ----------
        # Engines run their queues in order, so the PE stream must never
        # queue o(i) (which waits on chunk i's softmax chain) ahead of
        # independent work.  Per iteration i the PE sees:
        #   proj/vT(i) | outproj(i-2) | scores(i) | o(i-1)
        # which keeps it busy while softmax(i-1) runs on ACT/DVE.
        chunks = [(b, h) for b in range(B) for h in range(HS)]
        n = len(chunks)
        st = {}  # chunk index -> dict of live tiles

        def stage_load_proj(i):
            b, h = chunks[i]
            s = st[i] = {}
            xt = s["xt"] = xtp.tile([C + 1, FD], BF16, tag="xt", name="xt")
            nc.sync.dma_start(out=xt[:, :], in_=xt_d[b, h, :, :])
            xb = s["xb"] = xbp.tile([128, 1024], BF16, tag="xb", name="xb")
            nc.sync.dma_start(out=xb[:, :], in_=xb_d[b, h, :, :])

            # A/B PSUM banks: cols 0-255 scoresT, 256-511 vT.  Same bank
            # from different tile columns is fine; the parity split keeps
            # same-column matmuls in alternating banks.
            tS = ab_ps.tile([128, 512], F32, tag="A", name="tS")
            s["tAB"] = tAB = (tS, tS)
            tU = s["tU"] = (ab_ps.tile([128, 512], F32, tag="V", name="tU0"),
                            ab_ps.tile([128, 512], F32, tag="V2",
                                       name="tU1"))

            def ut_mm(p):
                # uT for site pair (2p, 2p+1): [64(2-site i), 64(c)] block
                # at partition half p%2, free slot (p%16)//2, tile p//16;
                # tile_position col 64*(p%2) (proven pattern).
                nc.tensor.matmul(
                    tU[p // 16][64 * (p % 2):64 * (p % 2) + 64,
                                64 * ((p % 16) // 2):
                                64 * ((p % 16) // 2) + 64],
                    xt[0:C, 64 * p:64 * p + 64],
                    wu_sb[:, :], start=True, stop=True,
                    tile_position=(0, 64 * (p % 2)))
            s["ut_mm"] = ut_mm

            # P2 = G~ @ X [65, 512] x4, interleaved with vT = X^T Wv^T
            # matmuls to space same-column same-bank pairs apart.
            pdr = s["pdr"] = []
            for qd in range(4):
                pp = pj_ps.tile([C + 1, 512], F32, tag="pp", name="pp")
                nc.tensor.matmul(pp[:, :], L_sb[:, :],
                                 xt[0:C, 512 * qd:512 * qd + 512],
                                 start=True, stop=True, tile_position=(0, 0))
                ut_mm(2 * qd)
                ut_mm(2 * qd + 1)
                pd = pdrp.tile([C + 1, 512], BF16, tag=f"pd{qd}", name="pd")
                _copy(nc, pd_eng[qd], pd[:, :], pp[:, :])
                pdr.append(pd)

        def stage_scores_softmax(i):
            s = st[i]
            xt, pdr, tAB = s["xt"], s["pdr"], s["tAB"]
            # scoresT: site w=4t+bc -> parity bank by t%2, partitions 32*bc,
            # free 32*(t//2); remaining vT matmuls spaced among the scores.
            for ti, t in enumerate(range(16)):
                sc = tAB[t % 2]
                fo = 32 * t
                # remaining 24 uT pair-matmuls spaced among the scores
                for p in range(8 + (24 * ti) // 16,
                               8 + (24 * (ti + 1)) // 16):
                    s["ut_mm"](p)
                for bc in range(4):
                    w = 4 * t + bc
                    nc.tensor.matmul(
                        sc[32 * bc:32 * bc + 32, fo:fo + 32],
                        xt[0:C + 1, 32 * w:32 * w + 32],
                        pdr[w // 16][0:C + 1,
                                     32 * (w % 16):32 * (w % 16) + 32],
                        start=True, stop=True,
                        tile_position=(0, 32 * bc))

            utdr = s["utdr"] = (
                sbp.tile([128, 512], BF16, tag="ut0", name="ut0"),
                sbp.tile([128, 512], BF16, tag="ut1", name="ut1"))
            _copy(nc, vtsb_eng[0], utdr[0][:, :], s["tU"][0][:, :])
            _copy(nc, vtsb_eng[1], utdr[1][:, :], s["tU"][1][:, :])
            e_sb = sbp.tile([128, 512], BF16, tag="e", name="e_sb")
            nc.scalar.activation(e_sb[:, :], tAB[0][:, :], EXP)
            den = sbp.tile([128, 16], F32, tag="den", name="den")
            nc.vector.reduce_sum(
                out=den[:, :],
                in_=mkap(e_sb, 0, 128, 0, [[32, 16], [1, 32]]),
                axis=mybir.AxisListType.X)
            atn = sbp.tile([128, 512], BF16, tag="atn", name="atn")
            if norm_eng == "div":
                nc.vector.tensor_tensor(
                    out=atn[:, :], in0=e_sb[:, :],
                    in1=mkap(den, 0, 128, 0, [[1, 16], [0, 32]]),
                    op=mybir.AluOpType.divide)
            elif norm_eng == "pooldiv":
                nc.gpsimd.tensor_tensor(
                    out=atn[:, :], in0=e_sb[:, :],
                    in1=mkap(den, 0, 128, 0, [[1, 16], [0, 32]]),
                    op=mybir.AluOpType.divide)
            else:
                rcp = sbp.tile([128, 16], F32, tag="rcp", name="rcp")
                nc.vector.reciprocal(rcp[:, :], den[:, :])
                neng = nc.gpsimd if norm_eng == "pool" else nc.vector
                neng.tensor_tensor(
                    out=atn[:, :], in0=e_sb[:, :],
                    in1=mkap(rcp, 0, 128, 0, [[1, 16], [0, 32]]),
                    op=mybir.AluOpType.mult)
            a4 = s["a4"] = sbp.tile([128, 512], BF16, tag="a4", name="a4")
            nc.vector.transpose(a4[:, :], atn[:, :])

        def stage_o(i):
            # z = (Wo Wv) X a straight into the y PSUM pair:
            # site w -> yp[(w%4)//2][64*(w%2)+c, 32*(w//4)+j]
            s = st[i]
            utdr, a4 = s["utdr"], s["a4"]
            yp = s["yp"] = (y_ps.tile([128, 512], F32, tag="y0", name="yp0"),
                            y_ps.tile([128, 512], F32, tag="y1", name="yp1"))
            for t in range(16):
                for bc in range(4):
                    w = 4 * t + bc
                    pb = 32 * (w % 4)
                    nc.tensor.matmul(
                        yp[(w % 4) // 2][64 * (w % 2):64 * (w % 2) + 64,
                                         32 * t:32 * t + 32],
                        utdr[t // 8][pb:pb + 32,
                                     64 * (t % 8):64 * (t % 8) + 64],
                        a4[pb:pb + 32, 32 * t:32 * t + 32],
                        start=True, stop=True,
                        tile_position=(pb, 64 * (w % 2)))

        def stage_out(i):
            b, h = chunks[i]
            s = st[i]
            xb = s["xb"]
            yp0, yp1 = s["yp"]
            y_sb = ysp.tile([128, 1024], ydt, tag="y", name="y_sb")
            _add(nc, fin_eng[0], y_sb[:, 0:512], yp0[:, :], xb[:, 0:512])
            _add(nc, fin_eng[1], y_sb[:, 512:1024], yp1[:, :],
                 xb[:, 512:1024])
            nc.sync.dma_start(out=y_d[b, h, :, :], in_=y_sb[:, :])
            del st[i]

        for i in range(n + 2):
            if i < n:
                stage_load_proj(i)
            if i >= 2:
                stage_out(i - 2)
            if i < n:
                stage_scores_softmax(i)
            if i >= 1 and i - 1 < n:
                stage_o(i - 1)

    nc.finalize()
    return nc


_NC_CACHE = {}


def get_nc():
    if "nc" not in _NC_CACHE:
        _NC_CACHE["nc"] = build_program()
    return _NC_CACHE["nc"]


def make_in_maps(x, Wk, bk, Wq, bq, Wv, bv, Wo, bo):
    f = np.float32
    bf = ml_dtypes.bfloat16
    x = np.asarray(x, f)
    Wk, bk = np.asarray(Wk, f), np.asarray(bk, f)
    Wq, bq = np.asarray(Wq, f), np.asarray(bq, f)
    Wv, bv = np.asarray(Wv, f), np.asarray(bv, f)
    Wo, bo = np.asarray(Wo, f), np.asarray(bo, f)

    isq = 1.0 / math.sqrt(S)
    G2 = (Wq.T @ Wk) * isq                     # [C, C]
    g = (Wk.T @ bq) * isq                      # [C]
    Gt = np.concatenate([G2, g[None, :]], 0)   # [C+1, C]
    L = np.ascontiguousarray(Gt.T).astype(bf)  # [C, C+1] lhsT
    wuT = np.ascontiguousarray((Wo @ Wv).T).astype(bf)   # [C, C]
    bst = Wo @ bv + bo                         # [C]

    # xt: [B, H, C+1, W*D] bf16, free index = 32*w + d, ones row at c=C
    xt_full = np.empty((B, H, C + 1, FD), dtype=bf)
    xw = np.transpose(x, (0, 3, 1, 4, 2))      # [B, H, C, W, D]
    xt_full[:, :, :C, :] = xw.reshape(B, H, C, FD).astype(bf)
    xt_full[:, :, C, :] = np.float32(1.0)

    # xb: [B, H, 128, 1024] bf16:
    #   partition 64*sg + c, free 512*hf + 32*t + d, site w = 4*t + 2*hf + sg
    xbv = x + bst[None, :, None, None, None]
    # [B, H, C, W, D] -> split w = 4*t + 2*hf + sg -> [B, H, sg, c, hf, t, d]
    xb6 = np.transpose(xbv, (0, 3, 1, 4, 2)).reshape(B, H, C, W // 4, 2, 2, D)
    # axes: b, h, c, t, hf, sg, d -> want [b, h, sg, c, hf, t, d]
    xb_full = np.ascontiguousarray(
        np.transpose(xb6, (0, 1, 5, 2, 4, 3, 6))).reshape(
            B, H, 128, 1024).astype(bf)

    in_maps = []
    for i in range(NCORES):
        sl = slice(i * HS, (i + 1) * HS)
        m = {
            "xt": np.ascontiguousarray(xt_full[:, sl]),
            "xb": np.ascontiguousarray(xb_full[:, sl]),
            "L": L, "wuT": wuT,
        }
        in_maps.append(m)
    return in_maps


def gather(results):
    out = np.empty((B, C, D, H, W), dtype=np.float32)
    for i in range(NCORES):
        yr = np.asarray(results[i]["y"], dtype=np.float32)  # [B,HS,128,1024]
        y7 = yr.reshape(B, HS, 2, 64, 2, 16, 32)  # b,h,sg,c,hf,t,d
        # -> [B, C, D, HS, t, hf, sg] then w = 4t + 2hf + sg
        yw = np.transpose(y7, (0, 3, 6, 1, 5, 4, 2)).reshape(
            B, 64, 32, HS, W)
        out[:, :, :, i * HS:(i + 1) * HS, :] = yw
    return out


def kernel(x, Wk, bk, Wq, bq, Wv, bv, Wo, bo):
    nc = get_nc()
    in_maps = make_in_maps(x, Wk, bk, Wq, bq, Wv, bv, Wo, bo)
    res = run_bass_kernel_spmd(nc, in_maps, core_ids=list(range(NCORES)))
    return gather(res.results)


# revision 37
# speedup vs baseline: 6.4263x; 1.0117x over previous
"""Trainium2 Bass kernel for 3D conv-attention layer (v2, bf16 + algebraic fusion).

Reference (per (b,h,w) "site", D=32 positions, S=32 features):
  k,q,v = 1x1 conv of x [B,C,D,H,W] -> [B,S,D,H,W]
  scoresT[j,i] = sum_s q[s,j] k[s,i] / sqrt(S)   (per site)
  aT = softmax over i  (free dim of scoresT)
  o[s,j] = sum_i v[s,i] a[i,j];   y = x + Wo @ o + bo

Key algebra (removes k/q projections AND the operand-colocation problem):
  scoresT/sqrt(S) = X~^T @ (G~ @ X)  per site, where
    G2 = Wq^T Wk / sqrt(S)  [C,C],  g = Wk^T bq / sqrt(S)  [C]
    G~ = [[G2],[g^T]] [C+1,C],  X~ = [X; ones] [C+1,D]
  (all j-only / const score terms cancel in the softmax over i;
   bv folds into a constant output bias since sum_i a[i,j] == 1:
   b* = Wo bv + bo, pre-added to x on the host.)

Cost-model-aware choices (TimelineSim charges matmuls by OUTPUT FREE SIZE
only, at 1 cycle/row for bf16; DVE/ACT ops by max free size):
  - everything bf16 into the PE, fp32 PSUM accumulation
  - P2 = G~X as 4 [65,512] matmuls/chunk (2048 rows)
  - vT = X^T Wv^T computed BY THE PE (16 matmuls, 4 sites each, 512 rows)
    -> no DVE StreamTranspose for v (StreamTranspose has no fast modes)
  - scores/o: 64 [32,32] matmuls each, stacked 4 sites per 128-partition
    PSUM tile so softmax runs as a few [128,512]-wide ops
  - single DVE block-transpose (aT -> a) per chunk
  - host pre-transposes x so every DMA is >=512B-contiguous per partition

Sharding: data-parallel over H (8 cores x 8 rows).
Per core: 32 chunks of (b,h), each 64 sites of [C=64, D=32].
"""

import math
from contextlib import ExitStack

import numpy as np
import ml_dtypes

import concourse.bass as bass
import concourse.mybir as mybir
from concourse import bacc
import concourse.tile as tile
from concourse.bass_utils import run_bass_kernel_spmd

B, C, D, H, W = 4, 64, 32, 64, 64
S = C // 2  # 32
NCORES = 8
HS = H // NCORES  # 8
F32 = mybir.dt.float32
BF16 = mybir.dt.bfloat16
FD = D * W  # 2048 free elems per chunk


def mkap(base, part0, pcount, foff, fdims):
    """AP at partition block [part0, part0+pcount) of a tile, free offset foff,
    free dims [(step, count), ...] in the tile's flat free space."""
    full = base[...] if not isinstance(base, bass.AP) else base
    pstride = full.ap[0][0]
    return bass.AP(tensor=full.tensor,
                   offset=full.offset + part0 * pstride + foff,
                   ap=[[pstride, pcount]] + [list(d) for d in fdims])


YDT = BF16  # output dtype (bf16 halves the store DMA)



def _copy(nc, eng, out, in_):
    if eng == "act":
        nc.scalar.activation(out, in_, mybir.ActivationFunctionType.Copy)
    elif eng == "dve":
        nc.vector.tensor_copy(out=out, in_=in_)
    else:
        nc.gpsimd.tensor_copy(out=out, in_=in_)


def _add(nc, eng, out, in0, in1):
    e = nc.vector if eng == "dve" else nc.gpsimd
    e.tensor_tensor(out=out, in0=in0, in1=in1, op=mybir.AluOpType.add)

def build_program(ydt=YDT, xt_bufs=4, xb_bufs=3, pdr_bufs=2, sb_bufs=4,
                  y_bufs=3, pj_bufs=2, vt_bufs=1, ab_bufs=1, yps_bufs=1,
                  probe=(), pd_eng=("act", "act", "act", "act"),
                  vtsb_eng=("act", "dve"), odr_eng=("act", "dve"),
                  fin_eng=("dve", "dve"), norm_eng="dve",
                  sc_order="tmaj", layout="single"):
    nc = bacc.Bacc()
    xt_d = nc.declare_dram_parameter("xt", [B, HS, C + 1, FD], BF16,
                                     isOutput=False)
    xb_d = nc.declare_dram_parameter("xb", [B, HS, 128, 1024], BF16,
                                     isOutput=False)
    L_d = nc.declare_dram_parameter("L", [C, C + 1], BF16, isOutput=False)
    wu_d = nc.declare_dram_parameter("wuT", [C, C], BF16, isOutput=False)
    y_d = nc.declare_dram_parameter("y", [B, HS, 128, 1024], ydt,
                                    isOutput=True)

    EXP = mybir.ActivationFunctionType.Exp
    CPY = mybir.ActivationFunctionType.Copy

    with tile.TileContext(nc) as tc, ExitStack() as ctx:
        const = ctx.enter_context(tc.tile_pool(name="const", bufs=1))
        xtp = ctx.enter_context(tc.tile_pool(name="xtp", bufs=xt_bufs))
        xbp = ctx.enter_context(tc.tile_pool(name="xbp", bufs=xb_bufs))
        pdrp = ctx.enter_context(tc.tile_pool(name="pdrp", bufs=pdr_bufs))
        sbp = ctx.enter_context(tc.tile_pool(name="sbp", bufs=sb_bufs))
        ysp = ctx.enter_context(tc.tile_pool(name="ysp", bufs=y_bufs))
        if layout == "single":
            pj_bufs = max(pj_bufs, 3)
        pj_ps = ctx.enter_context(tc.tile_pool(name="pj_ps", bufs=pj_bufs,
                                               space="PSUM"))
        ab_ps = ctx.enter_context(tc.tile_pool(name="ab_ps", bufs=ab_bufs,
                                               space="PSUM"))
        o_ps = ctx.enter_context(tc.tile_pool(name="o_ps", bufs=vt_bufs,
                                              space="PSUM"))
        y_ps = ctx.enter_context(tc.tile_pool(name="y_ps", bufs=yps_bufs,
                                              space="PSUM"))

        # ---- constants ----
        L_sb = const.tile([C, C + 1], BF16, tag="L")
        nc.sync.dma_start(out=L_sb[:, :], in_=L_d[:, :])
        wu_sb = const.tile([C, C], BF16, tag="wu")
        nc.sync.dma_start(out=wu_sb[:, :], in_=wu_d[:, :])

        # ---------- software-pipelined chunk emission ----------
        # Engines run their queues in order, so the PE stream must never
        # queue o(i) (which waits on chunk i's softmax chain) ahead of
        # independent work.  Per iteration i the PE sees:
        #   proj/vT(i) | outproj(i-2) | scores(i) | o(i-1)
        # which keeps it busy while softmax(i-1) runs on ACT/DVE.
        chunks = [(b, h) for b in range(B) for h in range(HS)]
        n = len(chunks)
        st = {}  # chunk index -> dict of live tiles

        def stage_load_proj(i):
            b, h = chunks[i]
            s = st[i] = {}
            xt = s["xt"] = xtp.tile([C + 1, FD], BF16, tag="xt", name="xt")
            nc.sync.dma_start(out=xt[:, :], in_=xt_d[b, h, :, :])
            xb = s["xb"] = xbp.tile([128, 1024], BF16, tag="xb", name="xb")
            nc.sync.dma_start(out=xb[:, :], in_=xb_d[b, h, :, :])

            # A/B PSUM banks: cols 0-255 scoresT, 256-511 vT.  Same bank
            # from different tile columns is fine; the parity split keeps
            # same-column matmuls in alternating banks.
            tS = ab_ps.tile([128, 512], F32, tag="A", name="tS")
            s["tAB"] = tAB = (tS, tS)
            tU = s["tU"] = (ab_ps.tile([128, 512], F32, tag="V", name="tU0"),
                            ab_ps.tile([128, 512], F32, tag="V2",
                                       name="tU1"))

            def ut_mm(p):
                # uT for site pair (2p, 2p+1): [64(2-site i), 64(c)] block
                # at partition half p%2, free slot (p%16)//2, tile p//16;
                # tile_position col 64*(p%2) (proven pattern).
                nc.tensor.matmul(
                    tU[p // 16][64 * (p % 2):64 * (p % 2) + 64,
                                64 * ((p % 16) // 2):
                                64 * ((p % 16) // 2) + 64],
                    xt[0:C, 64 * p:64 * p + 64],
                    wu_sb[:, :], start=True, stop=True,
                    tile_position=(0, 64 * (p % 2)))
            s["ut_mm"] = ut_mm

            # P2 = G~ @ X [65, 512] x4, interleaved with vT = X^T Wv^T
            # matmuls to space same-column same-bank pairs apart.
            pdr = s["pdr"] = []
            for qd in range(4):
                pp = pj_ps.tile([C + 1, 512], F32, tag="pp", name="pp")
                nc.tensor.matmul(pp[:, :], L_sb[:, :],
                                 xt[0:C, 512 * qd:512 * qd + 512],
                                 start=True, stop=True, tile_position=(0, 0))
                ut_mm(2 * qd)
                ut_mm(2 * qd + 1)
                pd = pdrp.tile([C + 1, 512], BF16, tag=f"pd{qd}", name="pd")
                _copy(nc, pd_eng[qd], pd[:, :], pp[:, :])
                pdr.append(pd)

        def stage_scores_softmax(i):
            s = st[i]
            xt, pdr, tAB = s["xt"], s["pdr"], s["tAB"]
            # scoresT: site w=4t+bc -> parity bank by t%2, partitions 32*bc,
            # free 32*(t//2); remaining vT matmuls spaced among the scores.
            for ti, t in enumerate(range(16)):
                sc = tAB[t % 2]
                fo = 32 * t
                # remaining 24 uT pair-matmuls spaced among the scores
                for p in range(8 + (24 * ti) // 16,
                               8 + (24 * (ti + 1)) // 16):
                    s["ut_mm"](p)
                for bc in range(4):
                    w = 4 * t + bc
                    nc.tensor.matmul(
                        sc[32 * bc:32 * bc + 32, fo:fo + 32],
                        xt[0:C + 1, 32 * w:32 * w + 32],
                        pdr[w // 16][0:C + 1,
                                     32 * (w % 16):32 * (w % 16) + 32],
                        start=True, stop=True,
                        tile_position=(0, 32 * bc))

            utdr = s["utdr"] = (
                sbp.tile([128, 512], BF16, tag="ut0", name="ut0"),
                sbp.tile([128, 512], BF16, tag="ut1", name="ut1"))
            _copy(nc, vtsb_eng[0], utdr[0][:, :], s["tU"][0][:, :])
            _copy(nc, vtsb_eng[1], utdr[1][:, :], s["tU"][1][:, :])
            e_sb = sbp.tile([128, 512], BF16, tag="e", name="e_sb")
            nc.scalar.activation(e_sb[:, :], tAB[0][:, :], EXP)
            den = sbp.tile([128, 16], F32, tag="den", name="den")
            nc.vector.reduce_sum(
                out=den[:, :],
                in_=mkap(e_sb, 0, 128, 0, [[32, 16], [1, 32]]),
                axis=mybir.AxisListType.X)
            atn = sbp.tile([128, 512], BF16, tag="atn", name="atn")
            if norm_eng == "div":
                nc.vector.tensor_tensor(
                    out=atn[:, :], in0=e_sb[:, :],
                    in1=mkap(den, 0, 128, 0, [[1, 16], [0, 32]]),
                    op=mybir.AluOpType.divide)
            elif norm_eng == "pooldiv":
                nc.gpsimd.tensor_tensor(
                    out=atn[:, :], in0=e_sb[:, :],
                    in1=mkap(den, 0, 128, 0, [[1, 16], [0, 32]]),
                    op=mybir.AluOpType.divide)
            else:
                rcp = sbp.tile([128, 16], F32, tag="rcp", name="rcp")
                nc.vector.reciprocal(rcp[:, :], den[:, :])
                neng = nc.gpsimd if norm_eng == "pool" else nc.vector
                neng.tensor_tensor(
                    out=atn[:, :], in0=e_sb[:, :],
                    in1=mkap(rcp, 0, 128, 0, [[1, 16], [0, 32]]),
                    op=mybir.AluOpType.mult)
            a4 = s["a4"] = sbp.tile([128, 512], BF16, tag="a4", name="a4")
            nc.vector.transpose(a4[:, :], atn[:, :])

        def stage_o(i):
            # z = (Wo Wv) X a straight into the y PSUM pair:
            # site w -> yp[(w%4)//2][64*(w%2)+c, 32*(w//4)+j]
            s = st[i]
            utdr, a4 = s["utdr"], s["a4"]
            yp = s["yp"] = (y_ps.tile([128, 512], F32, tag="y0", name="yp0"),
                            y_ps.tile([128, 512], F32, tag="y1", name="yp1"))
            for t in range(16):
                for bc in range(4):
                    w = 4 * t + bc
                    pb = 32 * (w % 4)
                    nc.tensor.matmul(
                        yp[(w % 4) // 2][64 * (w % 2):64 * (w % 2) + 64,
                                         32 * t:32 * t + 32],
                        utdr[t // 8][pb:pb + 32,
                                     64 * (t % 8):64 * (t % 8) + 64],
                        a4[pb:pb + 32, 32 * t:32 * t + 32],
                        start=True, stop=True,
                        tile_position=(pb, 64 * (w % 2)))

        def stage_out(i):
            b, h = chunks[i]
            s = st[i]
            xb = s["xb"]
            yp0, yp1 = s["yp"]
            y_sb = ysp.tile([128, 1024], ydt, tag="y", name="y_sb")
            _add(nc, fin_eng[0], y_sb[:, 0:512], yp0[:, :], xb[:, 0:512])
            _add(nc, fin_eng[1], y_sb[:, 512:1024], yp1[:, :],
                 xb[:, 512:1024])
            nc.sync.dma_start(out=y_d[b, h, :, :], in_=y_sb[:, :])
            del st[i]

        for i in range(n + 2):
            if i < n:
                stage_load_proj(i)
            if i >= 2:
                stage_out(i - 2)
            if i < n:
                stage_scores_softmax(i)
            if i >= 1 and i - 1 < n:
                stage_o(i - 1)

    nc.finalize()
    return nc


_NC_CACHE = {}


def get_nc():
    if "nc" not in _NC_CACHE:
        _NC_CACHE["nc"] = build_program()
    return _NC_CACHE["nc"]


def make_in_maps(x, Wk, bk, Wq, bq, Wv, bv, Wo, bo):
    f = np.float32
    bf = ml_dtypes.bfloat16
    x = np.asarray(x, f)
    Wk, bk = np.asarray(Wk, f), np.asarray(bk, f)
    Wq, bq = np.asarray(Wq, f), np.asarray(bq, f)
    Wv, bv = np.asarray(Wv, f), np.asarray(bv, f)
    Wo, bo = np.asarray(Wo, f), np.asarray(bo, f)

    isq = 1.0 / math.sqrt(S)
    G2 = (Wq.T @ Wk) * isq                     # [C, C]
    g = (Wk.T @ bq) * isq                      # [C]
    Gt = np.concatenate([G2, g[None, :]], 0)   # [C+1, C]
    L = np.ascontiguousarray(Gt.T).astype(bf)  # [C, C+1] lhsT
    wuT = np.ascontiguousarray((Wo @ Wv).T).astype(bf)   # [C, C]
    bst = Wo @ bv + bo                         # [C]

    # xt: [B, H, C+1, W*D] bf16, free index = 32*w + d, ones row at c=C
    xt_full = np.empty((B, H, C + 1, FD), dtype=bf)
    xw = np.transpose(x, (0, 3, 1, 4, 2))      # [B, H, C, W, D]
    xt_full[:, :, :C, :] = xw.reshape(B, H, C, FD).astype(bf)
    xt_full[:, :, C, :] = np.float32(1.0)

    # xb: [B, H, 128, 1024] bf16:
    #   partition 64*sg + c, free 512*hf + 32*t + d, site w = 4*t + 2*hf + sg
    xbv = x + bst[None, :, None, None, None]
    # [B, H, C, W, D] -> split w = 4*t + 2*hf + sg -> [B, H, sg, c, hf, t, d]
    xb6 = np.transpose(xbv, (0, 3, 1, 4, 2)).reshape(B, H, C, W // 4, 2, 2, D)
    # axes: b, h, c, t, hf, sg, d -> want [b, h, sg, c, hf, t, d]
    xb_full = np.ascontiguousarray(
        np.transpose(xb6, (0, 1, 5, 2, 4, 3, 6))).reshape(
            B, H, 128, 1024).astype(bf)

    in_maps = []
    for i in range(NCORES):
        sl = slice(i * HS, (i + 1) * HS)
        m = {
            "xt": np.ascontiguousarray(xt_full[:, sl]),
            "xb": np.ascontiguousarray(xb_full[:, sl]),
            "L": L, "wuT": wuT,
        }
        in_maps.append(m)
    return in_maps


def gather(results):
    out = np.empty((B, C, D, H, W), dtype=np.float32)
    for i in range(NCORES):
        yr = np.asarray(results[i]["y"], dtype=np.float32)  # [B,HS,128,1024]
        y7 = yr.reshape(B, HS, 2, 64, 2, 16, 32)  # b,h,sg,c,hf,t,d
        # -> [B, C, D, HS, t, hf, sg] then w = 4t + 2hf + sg
        yw = np.transpose(y7, (0, 3, 6, 1, 5, 4, 2)).reshape(
            B, 64, 32, HS, W)
        out[:, :, :, i * HS:(i + 1) * HS, :] = yw
    return out


def kernel(x, Wk, bk, Wq, bq, Wv, bv, Wo, bo):
    nc = get_nc()
    in_maps = make_in_maps(x, Wk, bk, Wq, bq, Wv, bv, Wo, bo)
    res = run_bass_kernel_spmd(nc, in_maps, core_ids=list(range(NCORES)))
    return gather(res.results)


# revision 43
# speedup vs baseline: 6.4465x; 1.0031x over previous
"""Trainium2 Bass kernel for 3D conv-attention layer (v2, bf16 + algebraic fusion).

Reference (per (b,h,w) "site", D=32 positions, S=32 features):
  k,q,v = 1x1 conv of x [B,C,D,H,W] -> [B,S,D,H,W]
  scoresT[j,i] = sum_s q[s,j] k[s,i] / sqrt(S)   (per site)
  aT = softmax over i  (free dim of scoresT)
  o[s,j] = sum_i v[s,i] a[i,j];   y = x + Wo @ o + bo

Key algebra (removes k/q projections AND the operand-colocation problem):
  scoresT/sqrt(S) = X~^T @ (G~ @ X)  per site, where
    G2 = Wq^T Wk / sqrt(S)  [C,C],  g = Wk^T bq / sqrt(S)  [C]
    G~ = [[G2],[g^T]] [C+1,C],  X~ = [X; ones] [C+1,D]
  (all j-only / const score terms cancel in the softmax over i;
   bv folds into a constant output bias since sum_i a[i,j] == 1:
   b* = Wo bv + bo, pre-added to x on the host.)

Cost-model-aware choices (TimelineSim charges matmuls by OUTPUT FREE SIZE
only, at 1 cycle/row for bf16; DVE/ACT ops by max free size):
  - everything bf16 into the PE, fp32 PSUM accumulation
  - P2 = G~X as 4 [65,512] matmuls/chunk (2048 rows)
  - Wo folded into v: uT = X^T (Wo Wv)^T computed BY THE PE (32 pair-
    matmuls, 2048 rows) -> z = uT^T a goes STRAIGHT into the y PSUM,
    eliminating the separate o matmuls, o drain, and out-projection
  - scores/z: 64 [32,32]/[64,32] matmuls each, stacked 4 sites per
    128-partition PSUM tile so softmax runs as [128,512]-wide ops
  - single DVE block-transpose (aT -> a) per chunk; no StreamTranspose
    for v/u (it has no fast DVE modes)
  - host pre-transposes x so every DMA is >=512B-contiguous per partition

Sharding: data-parallel over H (8 cores x 8 rows).
Per core: 32 chunks of (b,h), each 64 sites of [C=64, D=32].
"""

import math
from contextlib import ExitStack

import numpy as np
import ml_dtypes

import concourse.bass as bass
import concourse.mybir as mybir
from concourse import bacc
import concourse.tile as tile
from concourse.bass_utils import run_bass_kernel_spmd

B, C, D, H, W = 4, 64, 32, 64, 64
S = C // 2  # 32
NCORES = 8
HS = H // NCORES  # 8
F32 = mybir.dt.float32
BF16 = mybir.dt.bfloat16
FD = D * W  # 2048 free elems per chunk


def mkap(base, part0, pcount, foff, fdims):
    """AP at partition block [part0, part0+pcount) of a tile, free offset foff,
    free dims [(step, count), ...] in the tile's flat free space."""
    full = base[...] if not isinstance(base, bass.AP) else base
    pstride = full.ap[0][0]
    return bass.AP(tensor=full.tensor,
                   offset=full.offset + part0 * pstride + foff,
                   ap=[[pstride, pcount]] + [list(d) for d in fdims])


YDT = BF16  # output dtype (bf16 halves the store DMA)



def _copy(nc, eng, out, in_):
    if eng == "act":
        nc.scalar.activation(out, in_, mybir.ActivationFunctionType.Copy)
    elif eng == "dve":
        nc.vector.tensor_copy(out=out, in_=in_)
    else:
        nc.gpsimd.tensor_copy(out=out, in_=in_)


def _add(nc, eng, out, in0, in1):
    e = nc.vector if eng == "dve" else nc.gpsimd
    e.tensor_tensor(out=out, in0=in0, in1=in1, op=mybir.AluOpType.add)

def build_program(ydt=YDT, xt_bufs=4, xb_bufs=3, pdr_bufs=2, sb_bufs=4,
                  y_bufs=3, pj_bufs=2, vt_bufs=1, ab_bufs=1, yps_bufs=1,
                  probe=(), pd_eng=("act", "act", "act", "act"),
                  vtsb_eng=("act", "dve"), odr_eng=("act", "dve"),
                  fin_eng=("dve", "dve"), norm_eng="dve",
                  sc_order="tmaj", layout="single", order="lsoo"):
    nc = bacc.Bacc()
    xt_d = nc.declare_dram_parameter("xt", [B, HS, C + 1, FD], BF16,
                                     isOutput=False)
    xb_d = nc.declare_dram_parameter("xb", [B, HS, 128, 1024], BF16,
                                     isOutput=False)
    L_d = nc.declare_dram_parameter("L", [C, C + 1], BF16, isOutput=False)
    wu_d = nc.declare_dram_parameter("wuT", [C, C], BF16, isOutput=False)
    y_d = nc.declare_dram_parameter("y", [B, HS, 128, 1024], ydt,
                                    isOutput=True)

    EXP = mybir.ActivationFunctionType.Exp
    CPY = mybir.ActivationFunctionType.Copy

    with tile.TileContext(nc) as tc, ExitStack() as ctx:
        const = ctx.enter_context(tc.tile_pool(name="const", bufs=1))
        xtp = ctx.enter_context(tc.tile_pool(name="xtp", bufs=xt_bufs))
        xbp = ctx.enter_context(tc.tile_pool(name="xbp", bufs=xb_bufs))
        pdrp = ctx.enter_context(tc.tile_pool(name="pdrp", bufs=pdr_bufs))
        sbp = ctx.enter_context(tc.tile_pool(name="sbp", bufs=sb_bufs))
        ysp = ctx.enter_context(tc.tile_pool(name="ysp", bufs=y_bufs))
        if layout == "single":
            pj_bufs = max(pj_bufs, 3)
        pj_ps = ctx.enter_context(tc.tile_pool(name="pj_ps", bufs=pj_bufs,
                                               space="PSUM"))
        ab_ps = ctx.enter_context(tc.tile_pool(name="ab_ps", bufs=ab_bufs,
                                               space="PSUM"))
        o_ps = ctx.enter_context(tc.tile_pool(name="o_ps", bufs=vt_bufs,
                                              space="PSUM"))
        y_ps = ctx.enter_context(tc.tile_pool(name="y_ps", bufs=yps_bufs,
                                              space="PSUM"))

        # ---- constants ----
        L_sb = const.tile([C, C + 1], BF16, tag="L")
        nc.sync.dma_start(out=L_sb[:, :], in_=L_d[:, :])
        wu_sb = const.tile([C, C], BF16, tag="wu")
        nc.sync.dma_start(out=wu_sb[:, :], in_=wu_d[:, :])

        # ---------- software-pipelined chunk emission ----------
        # Engines run their queues in order, so the PE stream must never
        # queue o(i) (which waits on chunk i's softmax chain) ahead of
        # independent work.  Per iteration i the PE sees:
        #   proj/vT(i) | outproj(i-2) | scores(i) | o(i-1)
        # which keeps it busy while softmax(i-1) runs on ACT/DVE.
        chunks = [(b, h) for b in range(B) for h in range(HS)]
        n = len(chunks)
        st = {}  # chunk index -> dict of live tiles

        def stage_load_proj(i):
            b, h = chunks[i]
            s = st[i] = {}
            xt = s["xt"] = xtp.tile([C + 1, FD], BF16, tag="xt", name="xt")
            nc.sync.dma_start(out=xt[:, :], in_=xt_d[b, h, :, :])
            xb = s["xb"] = xbp.tile([128, 1024], BF16, tag="xb", name="xb")
            nc.sync.dma_start(out=xb[:, :], in_=xb_d[b, h, :, :])

            # A/B PSUM banks: cols 0-255 scoresT, 256-511 vT.  Same bank
            # from different tile columns is fine; the parity split keeps
            # same-column matmuls in alternating banks.
            tS = ab_ps.tile([128, 512], F32, tag="A", name="tS")
            s["tAB"] = tAB = (tS, tS)
            tU = s["tU"] = (ab_ps.tile([128, 512], F32, tag="V", name="tU0"),
                            ab_ps.tile([128, 512], F32, tag="V2",
                                       name="tU1"))

            def ut_mm(p):
                # uT for site pair (2p, 2p+1): [64(2-site i), 64(c)] block
                # at partition half p%2, free slot (p%16)//2, tile p//16;
                # tile_position col 64*(p%2) (proven pattern).
                nc.tensor.matmul(
                    tU[p // 16][64 * (p % 2):64 * (p % 2) + 64,
                                64 * ((p % 16) // 2):
                                64 * ((p % 16) // 2) + 64],
                    xt[0:C, 64 * p:64 * p + 64],
                    wu_sb[:, :], start=True, stop=True,
                    tile_position=(0, 64 * (p % 2)))
            s["ut_mm"] = ut_mm

            # P2 = G~ @ X [65, 512] x4, interleaved with vT = X^T Wv^T
            # matmuls to space same-column same-bank pairs apart.
            pdr = s["pdr"] = []
            for qd in range(4):
                pp = pj_ps.tile([C + 1, 512], F32, tag="pp", name="pp")
                nc.tensor.matmul(pp[:, :], L_sb[:, :],
                                 xt[0:C, 512 * qd:512 * qd + 512],
                                 start=True, stop=True, tile_position=(0, 0))
                ut_mm(2 * qd)
                ut_mm(2 * qd + 1)
                pd = pdrp.tile([C + 1, 512], BF16, tag=f"pd{qd}", name="pd")
                _copy(nc, pd_eng[qd], pd[:, :], pp[:, :])
                pdr.append(pd)

        def stage_scores_softmax(i):
            s = st[i]
            xt, pdr, tAB = s["xt"], s["pdr"], s["tAB"]
            # scoresT: site w=4t+bc -> parity bank by t%2, partitions 32*bc,
            # free 32*(t//2); remaining vT matmuls spaced among the scores.
            for ti, t in enumerate(range(16)):
                sc = tAB[t % 2]
                fo = 32 * t
                # remaining 24 uT pair-matmuls spaced among the scores
                for p in range(8 + (24 * ti) // 16,
                               8 + (24 * (ti + 1)) // 16):
                    s["ut_mm"](p)
                for bc in range(4):
                    w = 4 * t + bc
                    nc.tensor.matmul(
                        sc[32 * bc:32 * bc + 32, fo:fo + 32],
                        xt[0:C + 1, 32 * w:32 * w + 32],
                        pdr[w // 16][0:C + 1,
                                     32 * (w % 16):32 * (w % 16) + 32],
                        start=True, stop=True,
                        tile_position=(0, 32 * bc))

            utdr = s["utdr"] = (
                sbp.tile([128, 512], BF16, tag="ut0", name="ut0"),
                sbp.tile([128, 512], BF16, tag="ut1", name="ut1"))
            _copy(nc, vtsb_eng[0], utdr[0][:, :], s["tU"][0][:, :])
            _copy(nc, vtsb_eng[1], utdr[1][:, :], s["tU"][1][:, :])
            e_sb = sbp.tile([128, 512], BF16, tag="e", name="e_sb")
            nc.scalar.activation(e_sb[:, :], tAB[0][:, :], EXP)
            den = sbp.tile([128, 16], F32, tag="den", name="den")
            nc.vector.reduce_sum(
                out=den[:, :],
                in_=mkap(e_sb, 0, 128, 0, [[32, 16], [1, 32]]),
                axis=mybir.AxisListType.X)
            atn = sbp.tile([128, 512], BF16, tag="atn", name="atn")
            if norm_eng == "div":
                nc.vector.tensor_tensor(
                    out=atn[:, :], in0=e_sb[:, :],
                    in1=mkap(den, 0, 128, 0, [[1, 16], [0, 32]]),
                    op=mybir.AluOpType.divide)
            elif norm_eng == "pooldiv":
                nc.gpsimd.tensor_tensor(
                    out=atn[:, :], in0=e_sb[:, :],
                    in1=mkap(den, 0, 128, 0, [[1, 16], [0, 32]]),
                    op=mybir.AluOpType.divide)
            else:
                rcp = sbp.tile([128, 16], F32, tag="rcp", name="rcp")
                nc.vector.reciprocal(rcp[:, :], den[:, :])
                neng = nc.gpsimd if norm_eng == "pool" else nc.vector
                neng.tensor_tensor(
                    out=atn[:, :], in0=e_sb[:, :],
                    in1=mkap(rcp, 0, 128, 0, [[1, 16], [0, 32]]),
                    op=mybir.AluOpType.mult)
            a4 = s["a4"] = sbp.tile([128, 512], BF16, tag="a4", name="a4")
            nc.vector.transpose(a4[:, :], atn[:, :])

        def stage_o(i):
            # z = (Wo Wv) X a straight into the y PSUM pair:
            # site w -> yp[(w%4)//2][64*(w%2)+c, 32*(w//4)+j]
            s = st[i]
            utdr, a4 = s["utdr"], s["a4"]
            yp = s["yp"] = (y_ps.tile([128, 512], F32, tag="y0", name="yp0"),
                            y_ps.tile([128, 512], F32, tag="y1", name="yp1"))
            for t in range(16):
                for bc in range(4):
                    w = 4 * t + bc
                    pb = 32 * (w % 4)
                    nc.tensor.matmul(
                        yp[(w % 4) // 2][64 * (w % 2):64 * (w % 2) + 64,
                                         32 * t:32 * t + 32],
                        utdr[t // 8][pb:pb + 32,
                                     64 * (t % 8):64 * (t % 8) + 64],
                        a4[pb:pb + 32, 32 * t:32 * t + 32],
                        start=True, stop=True,
                        tile_position=(pb, 64 * (w % 2)))

        def stage_out(i):
            b, h = chunks[i]
            s = st[i]
            xb = s["xb"]
            yp0, yp1 = s["yp"]
            y_sb = ysp.tile([128, 1024], ydt, tag="y", name="y_sb")
            _add(nc, fin_eng[0], y_sb[:, 0:512], yp0[:, :], xb[:, 0:512])
            _add(nc, fin_eng[1], y_sb[:, 512:1024], yp1[:, :],
                 xb[:, 512:1024])
            nc.sync.dma_start(out=y_d[b, h, :, :], in_=y_sb[:, :])
            del st[i]

        for i in range(n + 2):
            if order == "loso":
                if i < n:
                    stage_load_proj(i)
                if i >= 2:
                    stage_out(i - 2)
                if i < n:
                    stage_scores_softmax(i)
                if i >= 1 and i - 1 < n:
                    stage_o(i - 1)
            elif order == "lsoo":
                if i < n:
                    stage_load_proj(i)
                if i < n:
                    stage_scores_softmax(i)
                if i >= 1 and i - 1 < n:
                    stage_o(i - 1)
                if i >= 2:
                    stage_out(i - 2)
            else:  # "olso"
                if i >= 1 and i - 1 < n:
                    stage_o(i - 1)
                if i < n:
                    stage_load_proj(i)
                if i >= 2:
                    stage_out(i - 2)
                if i < n:
                    stage_scores_softmax(i)

    nc.finalize()
    return nc


_NC_CACHE = {}


def get_nc():
    if "nc" not in _NC_CACHE:
        _NC_CACHE["nc"] = build_program()
    return _NC_CACHE["nc"]


def make_in_maps(x, Wk, bk, Wq, bq, Wv, bv, Wo, bo):
    f = np.float32
    bf = ml_dtypes.bfloat16
    x = np.asarray(x, f)
    Wk, bk = np.asarray(Wk, f), np.asarray(bk, f)
    Wq, bq = np.asarray(Wq, f), np.asarray(bq, f)
    Wv, bv = np.asarray(Wv, f), np.asarray(bv, f)
    Wo, bo = np.asarray(Wo, f), np.asarray(bo, f)

    isq = 1.0 / math.sqrt(S)
    G2 = (Wq.T @ Wk) * isq                     # [C, C]
    g = (Wk.T @ bq) * isq                      # [C]
    Gt = np.concatenate([G2, g[None, :]], 0)   # [C+1, C]
    L = np.ascontiguousarray(Gt.T).astype(bf)  # [C, C+1] lhsT
    wuT = np.ascontiguousarray((Wo @ Wv).T).astype(bf)   # [C, C]
    bst = Wo @ bv + bo                         # [C]

    # xt: [B, H, C+1, W*D] bf16, free index = 32*w + d, ones row at c=C
    xt_full = np.empty((B, H, C + 1, FD), dtype=bf)
    xw = np.transpose(x, (0, 3, 1, 4, 2))      # [B, H, C, W, D]
    xt_full[:, :, :C, :] = xw.reshape(B, H, C, FD).astype(bf)
    xt_full[:, :, C, :] = np.float32(1.0)

    # xb: [B, H, 128, 1024] bf16:
    #   partition 64*sg + c, free 512*hf + 32*t + d, site w = 4*t + 2*hf + sg
    xbv = x + bst[None, :, None, None, None]
    # [B, H, C, W, D] -> split w = 4*t + 2*hf + sg -> [B, H, sg, c, hf, t, d]
    xb6 = np.transpose(xbv, (0, 3, 1, 4, 2)).reshape(B, H, C, W // 4, 2, 2, D)
    # axes: b, h, c, t, hf, sg, d -> want [b, h, sg, c, hf, t, d]
    xb_full = np.ascontiguousarray(
        np.transpose(xb6, (0, 1, 5, 2, 4, 3, 6))).reshape(
            B, H, 128, 1024).astype(bf)

    in_maps = []
    for i in range(NCORES):
        sl = slice(i * HS, (i + 1) * HS)
        m = {
            "xt": np.ascontiguousarray(xt_full[:, sl]),
            "xb": np.ascontiguousarray(xb_full[:, sl]),
            "L": L, "wuT": wuT,
        }
        in_maps.append(m)
    return in_maps


def gather(results):
    out = np.empty((B, C, D, H, W), dtype=np.float32)
    for i in range(NCORES):
        yr = np.asarray(results[i]["y"], dtype=np.float32)  # [B,HS,128,1024]
        y7 = yr.reshape(B, HS, 2, 64, 2, 16, 32)  # b,h,sg,c,hf,t,d
        # -> [B, C, D, HS, t, hf, sg] then w = 4t + 2hf + sg
        yw = np.transpose(y7, (0, 3, 6, 1, 5, 4, 2)).reshape(
            B, 64, 32, HS, W)
        out[:, :, :, i * HS:(i + 1) * HS, :] = yw
    return out


def kernel(x, Wk, bk, Wq, bq, Wv, bv, Wo, bo):
    nc = get_nc()
    in_maps = make_in_maps(x, Wk, bk, Wq, bq, Wv, bv, Wo, bo)
    res = run_bass_kernel_spmd(nc, in_maps, core_ids=list(range(NCORES)))
    return gather(res.results)
